# revision 56
# baseline (speedup 1.0000x reference)
# Differential multi-head attention (dual softmax + GroupNorm + sigmoid gating)
# for Trainium2, batch-parallel across 8 NeuronCores (one batch row per core).
#
# Per-core math (batch b):
#   q = query @ Wq + bq -> per head: q1, q2, gate (each S x 64)
#   k = key   @ Wk + bk -> per head: k1, k2
#   v = values@ Wv + bv -> per head: v (S x 64)
#   attn = softmax(q1 k1^T / 8) - lam * softmax(q2 k2^T / 8)
#   out  = GroupNorm_{8 groups over d, reduced over (S, heads, d-in-group)}(attn @ v)
#   out  = out * (1 - lambda_init) * sigmoid(gate)
#
# Layout strategy: d-major ("transposed") attention: scores are computed as
# s^T (k on partitions, q free) so the attn@v contraction runs at K=128, and
# exp row-sums come free via a ones-column appended to v (M=65).  q1/q2 (and
# k1/k2) of each head live in complementary 64-partition halves of one tile;
# the two score matmuls of a head run as K=64 matmuls on disjoint partition
# halves (tile_position row 0 / 64), so no zero-padded key copies are needed.
# Matmul inputs are bf16 (single-pass PE); accumulation, softmax
# normalization and the GroupNorm statistics stay fp32.
#
# The gated output is algebraically refactored so the whole epilogue fuses
# into the output transposes:
#   out = (a[d]*y + ball[d]) * (tanh(g/2)+1)        (a,ball fold GN+lambda)
#       = a[d]*yg + ball[d]*bgt,   yg = y*(tanh+1), bgt = tanh+1
# yg/bgt are produced during the attention phase (bf16), and the final
# scale+transpose is two accumulating PE matmuls per 128-chunk against
# diag(a) / diag(ball) "scaled identity" matrices -- no post-stats vector
# pass over the full tensor remains.

import numpy as np

B, S_FULL, H, D = 8, 1024, 8, 64
DM = H * D  # 512


def build_nc(S=1024):
    import concourse.bacc as bacc
    import concourse.bass as bass
    import concourse.tile as tile
    from concourse import mybir
    from concourse.masks import make_identity

    f32 = mybir.dt.float32
    bf16 = mybir.dt.bfloat16
    AF = mybir.ActivationFunctionType
    OP = mybir.AluOpType
    AX = mybir.AxisListType

    NJ = S // 128          # k/seq 128-tiles
    CH = min(512, S)       # fp32-out matmul chunk
    NN = max(1, S // CH)
    CNT = float(S * H * (D // H))  # groupnorm reduction count per group
    EPS = 1e-3
    INV = 0.125            # 1/sqrt(64)

    nc = bacc.Bacc(target_bir_lowering=False)
    q_d = nc.dram_tensor("query", [S, DM], f32, kind="ExternalInput")
    k_d = nc.dram_tensor("key", [S, DM], f32, kind="ExternalInput")
    v_d = nc.dram_tensor("values", [S, DM], f32, kind="ExternalInput")
    wq_d = nc.dram_tensor("Wq", [DM, 3 * H * D], f32, kind="ExternalInput")
    bq_d = nc.dram_tensor("bq", [3 * H * D], f32, kind="ExternalInput")
    wk_d = nc.dram_tensor("Wk", [DM, 2 * H * D], f32, kind="ExternalInput")
    bk_d = nc.dram_tensor("bk", [2 * H * D], f32, kind="ExternalInput")
    wv_d = nc.dram_tensor("Wv", [DM, H * D], f32, kind="ExternalInput")
    bv_d = nc.dram_tensor("bv", [H * D], f32, kind="ExternalInput")
    gamma_d = nc.dram_tensor("gamma", [D], f32, kind="ExternalInput")
    beta_d = nc.dram_tensor("beta", [D], f32, kind="ExternalInput")
    lam_d = nc.dram_tensor("lam", [1], f32, kind="ExternalInput")
    li_d = nc.dram_tensor("lambda_init", [1], f32, kind="ExternalInput")
    out_d = nc.dram_tensor("out", [S, DM], f32, kind="ExternalOutput")

    ts_ = nc.vector.tensor_scalar
    stt = nc.vector.scalar_tensor_tensor

    with tile.TileContext(nc) as tc:
        with tc.tile_pool(name="consts", bufs=1) as consts, \
             tc.tile_pool(name="persist", bufs=1) as persist:

            # bf16 identity FIRST on the gpsimd queue: the input transposes
            # depend on it, so nothing may precede it there.
            ident_b = consts.tile([128, 128], bf16, tag="ident_b", name="ident_b")
            make_identity(nc, ident_b)

            # gate-projection weight gather on SWDGE: issue right away so the
            # (slow) software DMA completes long before the gate matmuls.
            wgt = []
            for r in range(4):
                w_t = consts.tile([128, 512], bf16, tag=f"wg{r}", name=f"wg{r}")
                nc.gpsimd.dma_start(
                    out=w_t,
                    in_=wq_d[128 * r:128 * (r + 1), :].rearrange(
                        "k (h blk) -> k h blk", blk=192)[:, :, 128:192])
                wgt.append(w_t)

            # persistent projection outputs (bf16, d-major)
            # kk[h]: rows 0-63 = k1 of head h, rows 64-127 = k2.
            # qz1[h] rows 0-63 = q1 (rest 0), qz2[h] rows 64-127 = q2 (rest 0):
            # zero-padding on the q (moving) side keeps every score matmul a
            # uniform K=128/M=128 shape -- split-row-group (K=64) matmuls
            # trip the PE power governor into a sustained half-clock throttle.
            qz1 = [persist.tile([128, S], bf16, tag=f"qz1{h}", name=f"qz1{h}") for h in range(8)]
            qz2 = [persist.tile([128, S], bf16, tag=f"qz2{h}", name=f"qz2{h}") for h in range(8)]
            kk = [persist.tile([128, S], bf16, tag=f"kk{h}", name=f"kk{h}") for h in range(8)]
            # zero the pad halves on gpsimd (idle after the consts; keeps both
            # the DVE queue and the ACT epilogues unblocked)
            for h in range(8):
                nc.gpsimd.memset(qz1[h][64:128, :], 0.0)
                nc.gpsimd.memset(qz2[h][0:64, :], 0.0)
            # gate stays head-pair packed: gt[p] rows 0-63 = head 2p, 64-127 = 2p+1
            gt = [persist.tile([128, S], bf16, tag=f"gt{p}", name=f"gt{p}") for p in range(4)]
            va = [persist.tile([128, 8, 65], bf16, tag=f"va{i}", name=f"va{i}") for i in range(NJ)]
            # ypair holds yg = y*(tanh(g/2)+1); bgt holds tanh(g/2)+1 (both bf16)
            ypair = [persist.tile([128, S], bf16, tag=f"yp{p}", name=f"yp{p}") for p in range(4)]
            bgt = [persist.tile([128, S], bf16, tag=f"bgt{p}", name=f"bgt{p}") for p in range(4)]
            sumcol = persist.tile([64, 16], f32, tag="sumcol", name="sumcol")

            # ---------- phase 1: load + transpose inputs, projections ----------
            with tc.tile_pool(name="xin", bufs=3) as xin_pool, \
                 tc.tile_pool(name="xtp", bufs=1) as xtp, \
                 tc.tile_pool(name="wload", bufs=1) as wpool, \
                 tc.tile_pool(name="thp", bufs=2) as thp, \
                 tc.tile_pool(name="ps_in", bufs=1, space="PSUM") as ps_in, \
                 tc.tile_pool(name="ps_proj", bufs=4, space="PSUM") as ps_proj:

                GRP = min(4, NJ)
                # x^T tiles are shared across q/k/v (WAR deps serialize on PE
                # program order anyway; saves 16KB/partition of SBUF)
                xt = [xtp.tile([128, S], bf16, tag=f"xt{c}", name=f"xt{c}")
                      for c in range(4)]

                def transpose_input(x_dram, dst=None, mid=None, on_act=False):
                    # on_act: run the bf16 casts + PSUM drains on the scalar
                    # engine -- it is idle before the first projection
                    # epilogues, and this unblocks the PE transposes sooner
                    cp = nc.scalar.copy if on_act else nc.vector.tensor_copy
                    dst = dst if dst is not None else xt
                    tp_cur = [None] * 4
                    for i in range(NJ):
                        xs = xin_pool.tile([128, DM], f32, tag="xs", name="xs")
                        nc.sync.dma_start(out=xs, in_=x_dram[128 * i:128 * (i + 1), :])
                        xq = xin_pool.tile([128, DM], bf16, tag="xin", name="xin")
                        cp(xq, xs)
                        if i % GRP == 0:
                            for c in range(4):
                                tp_cur[c] = ps_in.tile(
                                    [128, 128 * GRP], bf16, tag=f"tp{c}", name=f"tp{c}")
                        for c in range(4):
                            nc.tensor.transpose(
                                tp_cur[c][:, 128 * (i % GRP):128 * (i % GRP + 1)],
                                xq[:, 128 * c:128 * (c + 1)], ident_b)
                        if i % GRP == GRP - 1:
                            base = 128 * GRP * (i // GRP)
                            for c in range(4):
                                cp(dst[c][:, base:base + 128 * GRP], tp_cur[c])
                        if mid is not None and i == GRP - 1:
                            mid()
                    return dst

                # fp32 weight staging (HWDGE) + DVE downcast; staging tiles are
                # shared q->k->v (WAR on the quick downcast, saves 24KB SBUF)
                wst = [wpool.tile([128, 3 * H * D], f32, tag=f"wst{r}", name=f"wst{r}")
                       for r in range(4)]
                wqf = [wpool.tile([128, 3 * H * D], bf16, tag=f"wqf{r}", name=f"wqf{r}") for r in range(4)]
                wkf = [wpool.tile([128, 2 * H * D], bf16, tag=f"wkf{r}", name=f"wkf{r}") for r in range(4)]
                wvf = [wpool.tile([128, H * D], bf16, tag=f"wvf{r}", name=f"wvf{r}") for r in range(4)]

                def stage_wq():
                    for r in range(4):
                        nc.sync.dma_start(out=wst[r], in_=wq_d[128 * r:128 * (r + 1), :])
                        nc.vector.tensor_copy(wqf[r], wst[r])

                # --- query path (its first DMAs lead the sync queue; the wq
                # staging is interleaved after the first transpose group) ---
                xtq = transpose_input(q_d, mid=stage_wq, on_act=True)
                bqp = consts.tile([128, 8], f32, tag="bqp", name="bqp")
                nc.sync.dma_start(
                    out=bqp,
                    in_=bq_d[:].rearrange("(h blk) -> blk h", blk=192)[0:128, :])
                bg = consts.tile([128, 4], f32, tag="bg", name="bg")
                bqv = bq_d[:].rearrange("(h blk) -> h blk", blk=192)
                for p in range(4):
                    nc.sync.dma_start(out=bg[:, p:p + 1],
                                      in_=bqv[2 * p:2 * p + 2, 128:192])
                for h in range(8):
                    for n in range(NN):
                        ps = ps_proj.tile([128, CH], f32, tag="proj", name="proj")
                        for r in range(4):
                            nc.tensor.matmul(
                                ps, wqf[r][:, 192 * h:192 * h + 128],
                                xtq[r][:, CH * n:CH * (n + 1)],
                                start=(r == 0), stop=(r == 3))
                        nc.scalar.activation(
                            qz1[h][0:64, CH * n:CH * (n + 1)], ps[0:64, :],
                            AF.Identity, bias=bqp[0:64, h:h + 1])
                        nc.scalar.activation(
                            qz2[h][64:128, CH * n:CH * (n + 1)], ps[64:128, :],
                            AF.Identity, bias=bqp[64:128, h:h + 1])
                for p in range(4):
                    for n in range(NN):
                        ps = ps_proj.tile([128, CH], f32, tag="proj", name="proj")
                        for r in range(4):
                            nc.tensor.matmul(
                                ps, wgt[r][:, 128 * p:128 * (p + 1)],
                                xtq[r][:, CH * n:CH * (n + 1)],
                                start=(r == 0), stop=(r == 3))
                        nc.scalar.activation(
                            gt[p][:, CH * n:CH * (n + 1)], ps, AF.Identity,
                            bias=bg[:, p:p + 1])

                # --- deferred scalar/stat constants (off the critical path) ---
                lam128 = consts.tile([128, 1], f32, tag="lam128", name="lam128")
                nc.gpsimd.dma_start(out=lam128, in_=lam_d[:].to_broadcast([128, 1]))
                li128 = consts.tile([128, 1], f32, tag="li128", name="li128")
                nc.gpsimd.dma_start(out=li128, in_=li_d[:].to_broadcast([128, 1]))
                neglam = consts.tile([128, 1], f32, tag="neglam", name="neglam")
                ts_(neglam, lam128, -1.0, None, OP.mult)
                onelam = consts.tile([128, 1], f32, tag="onelam", name="onelam")
                ts_(onelam, lam128, -1.0, 1.0, OP.mult, OP.add)   # 1 - lam
                halfli = consts.tile([128, 1], f32, tag="halfli", name="halfli")
                ts_(halfli, li128, -0.5, 0.5, OP.mult, OP.add)    # 0.5*(1-li)

                bkp = consts.tile([128, 8], f32, tag="bkp", name="bkp")
                nc.sync.dma_start(
                    out=bkp,
                    in_=bk_d[:].rearrange("(h blk) -> blk h", blk=128))

                gamma128 = consts.tile([128, 1], f32, tag="gamma128", name="gamma128")
                nc.sync.dma_start(out=gamma128[0:64, :], in_=gamma_d[:])
                nc.sync.dma_start(out=gamma128[64:128, :], in_=gamma_d[:])
                beta128 = consts.tile([128, 1], f32, tag="beta128", name="beta128")
                nc.sync.dma_start(out=beta128[0:64, :], in_=beta_d[:])
                nc.sync.dma_start(out=beta128[64:128, :], in_=beta_d[:])
                bb128 = consts.tile([128, 1], f32, tag="bb128", name="bb128")
                ts_(bb128, beta128, halfli, None, OP.mult)        # beta*0.5*(1-li)

                # v-bias columns: head-major [64,8] for the stats corrections,
                # pair-stacked [128,4] for the final affine
                bvc = consts.tile([64, 8], f32, tag="bvc", name="bvc")
                nc.sync.dma_start(
                    out=bvc, in_=bv_d[:].rearrange("(h d) -> d h", d=64))
                cc64 = consts.tile([64, 8], f32, tag="cc64", name="cc64")
                ts_(cc64, bvc, onelam[0:64, :], None, OP.mult)
                bvc128 = consts.tile([128, 4], f32, tag="bvc128", name="bvc128")
                nc.sync.dma_start(
                    out=bvc128, in_=bv_d[:].rearrange("(p k d) -> (k d) p", k=2, d=64))
                cc128 = consts.tile([128, 4], f32, tag="cc128", name="cc128")
                ts_(cc128, bvc128, onelam, None, OP.mult)
                # cc-only GroupNorm stat corrections (ready before the tail)
                csq64 = consts.tile([64, 1], f32, tag="csq64", name="csq64")
                csum64 = consts.tile([64, 1], f32, tag="csum64", name="csum64")
                ccsq = consts.tile([64, 8], f32, tag="ccsq", name="ccsq")
                nc.vector.tensor_mul(ccsq, cc64, cc64)
                nc.vector.tensor_reduce(csq64, ccsq, axis=AX.X, op=OP.add)
                nc.vector.tensor_reduce(csum64, cc64, axis=AX.X, op=OP.add)

                # group matrix for the stats matmul, duplicated across both
                # 64-row halves: ind2b[d, d'] = 1 iff d//8 == (d' mod 64)//8
                ind2b = consts.tile([64, 128], f32, tag="ind2b", name="ind2b")
                nc.gpsimd.memset(ind2b, 1.0)
                nc.gpsimd.affine_select(
                    out=ind2b, in_=ind2b, compare_op=OP.is_ge, fill=0.0,
                    base=0, pattern=[[0, 2], [-8, 8], [0, 8]], channel_multiplier=1)
                nc.gpsimd.affine_select(
                    out=ind2b, in_=ind2b, compare_op=OP.is_ge, fill=0.0,
                    base=7, pattern=[[0, 2], [8, 8], [0, 8]], channel_multiplier=-1)

                # selector for the last half's PE-broadcast of the softmax
                # normalizers: sel2[r, x] = 1 iff x//64 == r   (r in 0..1)
                # bf16 so the broadcast matmul runs at full (non-fp32) rate
                sel2 = consts.tile([2, 128], bf16, tag="sel2", name="sel2")
                nc.gpsimd.memset(sel2, 1.0)
                nc.gpsimd.affine_select(
                    out=sel2, in_=sel2, compare_op=OP.is_ge, fill=0.0,
                    base=0, pattern=[[1, 128]], channel_multiplier=-64)
                nc.gpsimd.affine_select(
                    out=sel2, in_=sel2, compare_op=OP.is_ge, fill=0.0,
                    base=63, pattern=[[-1, 128]], channel_multiplier=64)

                # --- key path ---
                xtk = transpose_input(k_d)
                for r in range(4):
                    nc.sync.dma_start(out=wst[r][:, 0:2 * H * D],
                                      in_=wk_d[128 * r:128 * (r + 1), :])
                    nc.vector.tensor_copy(wkf[r], wst[r][:, 0:2 * H * D])
                for h in range(8):
                    for n in range(NN):
                        ps = ps_proj.tile([128, CH], f32, tag="proj", name="proj")
                        for r in range(4):
                            nc.tensor.matmul(
                                ps, wkf[r][:, 128 * h:128 * (h + 1)],
                                xtk[r][:, CH * n:CH * (n + 1)],
                                start=(r == 0), stop=(r == 3))
                        nc.scalar.activation(
                            kk[h][:, CH * n:CH * (n + 1)], ps,
                            AF.Identity, bias=bkp[:, h:h + 1])

                # --- values path (q-major, interleaved into v_aug + ones) ---
                xtv = transpose_input(v_d)
                for r in range(4):
                    nc.sync.dma_start(out=wst[r][:, 0:H * D],
                                      in_=wv_d[128 * r:128 * (r + 1), :])
                    nc.vector.tensor_copy(wvf[r], wst[r][:, 0:H * D])
                for i in range(NJ):
                    ps = ps_proj.tile([128, 512], f32, tag="proj", name="proj")
                    for r in range(4):
                        nc.tensor.matmul(
                            ps, xtv[r][:, 128 * i:128 * (i + 1)], wvf[r],
                            start=(r == 0), stop=(r == 3))
                    nc.vector.tensor_copy(
                        va[i][:, :, 0:64],
                        ps.rearrange("p (h d) -> p h d", d=64))
                    nc.gpsimd.memset(va[i][:, :, 64:65], 1.0)

                # gate tanh now (ACT is free here); bgt = tanh(g/2) + 1  (bf16)
                for p in range(4):
                    th = thp.tile([128, S], f32, tag="th", name="th")
                    nc.scalar.activation(th, gt[p], AF.Tanh, scale=0.5)
                    ts_(bgt[p], th, 1.0, None, OP.add)



            # ---------- phase 2: attention per head (pairs for epilogue) ----
            with tc.tile_pool(name="ps_att", bufs=2, space="PSUM") as ps_att, \
                 tc.tile_pool(name="ps_o", bufs=2, space="PSUM") as ps_o, \
                 tc.tile_pool(name="expp", bufs=4) as expp, \
                 tc.tile_pool(name="osp", bufs=2) as osp, \
                 tc.tile_pool(name="typ", bufs=2) as typ, \
                 tc.tile_pool(name="tailp", bufs=1) as tailp, \
                 tc.tile_pool(name="oq", bufs=3) as oqp, \
                 tc.tile_pool(name="spp", bufs=2) as spp:

                ty3 = None
                for p in range(4):
                    ty = typ.tile([128, S], f32, tag="ty", name="ty")
                    for half in range(2):
                        h = 2 * p + half
                        last = (p == 3 and half == 1)
                        prow = 64 * half   # q1/k1 in rows 0-63, q2/k2 in 64-127
                        sp2 = spp.tile([2, S], f32, tag="sp2", name="sp2")
                        os_c = {}
                        # term-sequential: only one o accumulator lives at a
                        # time, so both the score tiles and the o tiles can
                        # double-buffer inside the 8-bank PSUM budget.
                        for t, qz_ in ((1, qz1), (2, qz2)):
                            o_ps = ps_o.tile([65, S], f32, tag="o", name="o")
                            for j in range(NJ):
                                s_ps = ps_att.tile([128, S], f32, tag="s", name="s")
                                for n in range(NN):
                                    nc.tensor.matmul(
                                        s_ps[:, CH * n:CH * (n + 1)],
                                        kk[h][:, 128 * j:128 * (j + 1)],
                                        qz_[h][:, CH * n:CH * (n + 1)],
                                        start=True, stop=True)
                                ex = expp.tile([128, S], bf16, tag="exp", name="exp")
                                nc.scalar.activation(ex, s_ps, AF.Exp, scale=INV)
                                for n in range(NN):
                                    nc.tensor.matmul(
                                        o_ps[:, CH * n:CH * (n + 1)],
                                        va[j][:, h, :],
                                        ex[:, CH * n:CH * (n + 1)],
                                        start=(j == 0), stop=(j == NJ - 1))
                            os_ = osp.tile([65, S], f32, tag=f"os{t}", name=f"os{t}")
                            if last:
                                # ACT is idle once the exps are done; freeing
                                # the DVE queue for the combine chain
                                nc.scalar.copy(os_, o_ps)
                            else:
                                nc.vector.tensor_copy(os_, o_ps)
                            nc.sync.dma_start(
                                out=sp2[t - 1:t, :], in_=os_[64:65, :])
                            os_c[t] = os_

                        # per-half combine: softmax-normalize, subtract the
                        # lam-weighted term, accumulate GN stats
                        rp2 = spp.tile([2, S], f32, tag="rp2", name="rp2")
                        if last:
                            # tail: fast reciprocal (~1e-3 rel err on two
                            # heads' normalizers, well inside tolerance) and
                            # a selector-matmul broadcast on the idle PE
                            nc.vector.reciprocal_approx_fast(out=rp2, in_=sp2)
                            rpb = spp.tile([2, S], bf16, tag="rpb", name="rpb")
                            nc.vector.tensor_copy(rpb, rp2)
                            bc = ps_att.tile([128, S], f32, tag="s", name="bcpe")
                            for n in range(NN):
                                nc.tensor.matmul(
                                    bc[:, CH * n:CH * (n + 1)], sel2,
                                    rpb[:, CH * n:CH * (n + 1)],
                                    start=True, stop=True)
                            bcs1, bcs2 = bc[0:64, :], bc[64:128, :]
                        else:
                            rscr = spp.tile([2, S], f32, tag="rscr", name="rscr")
                            nc.vector.reciprocal_approx_accurate(rp2, sp2, rscr)
                            bcs1 = spp.tile([64, S], f32, tag="bcs1", name="bcs1")
                            bcs2 = spp.tile([64, S], f32, tag="bcs2", name="bcs2")
                            r1 = spp.tile([1, S], f32, tag="rst1", name="rst1")
                            nc.sync.dma_start(out=r1, in_=rp2[0:1, :])
                            r2 = spp.tile([1, S], f32, tag="rst2", name="rst2")
                            nc.sync.dma_start(out=r2, in_=rp2[1:2, :])
                            nc.gpsimd.partition_broadcast(bcs1, r1[0:1, :], channels=64)
                            nc.gpsimd.partition_broadcast(bcs2, r2[0:1, :], channels=64)
                        os1, os2 = os_c[1], os_c[2]
                        nc.vector.tensor_mul(os1[0:64, :], os1[0:64, :], bcs1)
                        stt(os2[0:64, :], os2[0:64, :], neglam[0:64, :], bcs2,
                            OP.mult, OP.mult)
                        tyh = ty[prow:prow + 64, :]
                        stt(tyh, os1[0:64, :], 1.0, os2[0:64, :],
                            OP.bypass, OP.add, accum_out=sumcol[:, h:h + 1])
                        stt(os1[0:64, :], tyh, 1.0, tyh, OP.mult, OP.mult,
                            accum_out=sumcol[:, 8 + h:9 + h])
                    # yg = y * (tanh+1)  (bf16, consumed by the fused
                    # scale+transpose in phase 3).  The last pair's is
                    # deferred into the stats chain so the reductions start
                    # immediately after its ysq.
                    if p < 3:
                        nc.vector.tensor_mul(ypair[p], ty, bgt[p])
                    else:
                        ty3 = ty

                # ------- tail: stats + fused scale/transpose output -------
                # (same pool scope: ty3 must stay alive, and the stats/output
                # PSUM reuses the attention pools' banks)
                tot = tailp.tile([64, 2], f32, tag="tot", name="tot")
                nc.vector.tensor_reduce(
                    tot, sumcol.rearrange("p (t h) -> p t h", h=8),
                    axis=AX.X, op=OP.add)
                # bias-C (bv) corrections to the raw-Y stats
                csc = tailp.tile([64, 8], f32, tag="csc", name="csc")
                nc.vector.tensor_mul(csc, cc64, sumcol[:, 0:8])
                cy64 = tailp.tile([64, 1], f32, tag="cy64", name="cy64")
                nc.vector.tensor_reduce(cy64, csc, axis=AX.X, op=OP.add)
                tot2 = tailp.tile([64, 2], f32, tag="tot2", name="tot2")
                stt(tot2[:, 0:1], csum64, float(S), tot[:, 0:1], OP.mult, OP.add)
                stt(tot2[:, 1:2], cy64, 2.0, tot[:, 1:2], OP.mult, OP.add)
                stt(tot2[:, 1:2], csq64, float(S), tot2[:, 1:2], OP.mult, OP.add)

                ms_ps = ps_o.tile([128, 2], f32, tag="o", name="ms")
                nc.tensor.matmul(ms_ps, ind2b, tot2, start=True, stop=True)
                mean = tailp.tile([128, 1], f32, tag="mean", name="mean")
                ts_(mean, ms_ps[:, 0:1], 1.0 / CNT, None, OP.mult)
                e2 = tailp.tile([128, 1], f32, tag="e2", name="e2")
                ts_(e2, ms_ps[:, 1:2], 1.0 / CNT, None, OP.mult)
                nm2 = tailp.tile([128, 1], f32, tag="nm2", name="nm2")
                ts_(nm2, mean, mean, -1.0, OP.mult, OP.mult)
                veps = tailp.tile([128, 1], f32, tag="veps", name="veps")
                stt(veps, nm2, EPS, e2, OP.add, OP.add)
                # deferred last-pair gate fold: slots into the DVE queue while
                # the scalar engine loads the sqrt table set
                nc.vector.tensor_mul(ypair[3], ty3, bgt[3])
                sd = tailp.tile([128, 1], f32, tag="sd", name="sd")
                nc.scalar.activation(sd, veps, AF.Sqrt)
                rsd = tailp.tile([128, 1], f32, tag="rsd", name="rsd")
                nc.vector.reciprocal(rsd, sd)
                # one Newton step for rsqrt accuracy (ACT sqrt is loose)
                rr = tailp.tile([128, 1], f32, tag="rr", name="rr")
                nc.vector.tensor_mul(rr, rsd, rsd)
                nc.vector.tensor_mul(rr, rr, veps)
                ts_(rr, rr, -0.5, 1.5, OP.mult, OP.add)
                rstd = tailp.tile([128, 1], f32, tag="rstd", name="rstd")
                nc.vector.tensor_mul(rstd, rsd, rr)

                a128 = tailp.tile([128, 1], f32, tag="a128", name="a128")
                ts_(a128, rstd, gamma128, halfli, OP.mult, OP.mult)
                cm128 = tailp.tile([128, 4], f32, tag="cm128", name="cm128")
                ts_(cm128, cc128, mean, None, OP.subtract)
                ball = tailp.tile([128, 4], f32, tag="ball", name="ball")
                ts_(ball, cm128, a128, bb128, OP.mult, OP.add)

                # scaled-identity matrices: out = yg^T diag(a) + bgt^T diag(ball)
                da = tailp.tile([128, 128], bf16, tag="da", name="da")
                ts_(da, ident_b, a128, None, OP.mult)
                db = []
                for p in range(4):
                    d_t = tailp.tile([128, 128], bf16, tag=f"db{p}", name=f"db{p}")
                    ts_(d_t, ident_b, ball[:, p:p + 1], None, OP.mult)
                    db.append(d_t)

                for c in range(NJ):
                    tp_o = ps_att.tile([128, 512], f32, tag="s", name="tp_out")
                    for p in range(4):
                        nc.tensor.matmul(
                            tp_o[:, 128 * p:128 * (p + 1)],
                            ypair[p][:, 128 * c:128 * (c + 1)], da,
                            start=True, stop=False)
                        nc.tensor.matmul(
                            tp_o[:, 128 * p:128 * (p + 1)],
                            bgt[p][:, 128 * c:128 * (c + 1)], db[p],
                            start=False, stop=True)
                    oq = oqp.tile([128, 512], f32, tag="oq", name="oq")
                    nc.scalar.copy(oq, tp_o)
                    nc.sync.dma_start(out=out_d[128 * c:128 * (c + 1), :], in_=oq)

    nc.finalize()
    return nc


_CACHE = {}


def _get_nc():
    if "nc" not in _CACHE:
        _CACHE["nc"] = build_nc(S_FULL)
    return _CACHE["nc"]


def run(inputs, trace=False, tmpdir=None):
    from concourse.bass_utils import run_bass_kernel_spmd
    nc = _get_nc()
    arrs = {k: np.asarray(v, dtype=np.float32) for k, v in inputs.items()}
    shared = {k: np.ascontiguousarray(arrs[k]) for k in
              ("Wq", "bq", "Wk", "bk", "Wv", "bv", "gamma", "beta",
               "lam", "lambda_init")}
    in_maps = []
    for i in range(B):
        m = dict(shared)
        m["query"] = np.ascontiguousarray(arrs["query"][i])
        m["key"] = np.ascontiguousarray(arrs["key"][i])
        m["values"] = np.ascontiguousarray(arrs["values"][i])
        in_maps.append(m)
    res = run_bass_kernel_spmd(nc, in_maps, core_ids=list(range(B)),
                               trace=trace, tmpdir=tmpdir)
    out = np.stack([res.results[i]["out"] for i in range(B)], axis=0)
    return out.astype(np.float32), res


def kernel(**inputs):
    out, _ = run(inputs)
    return out


# revision 57
# speedup vs baseline: 1.1657x; 1.1657x over previous
# Differential multi-head attention (dual softmax + GroupNorm + sigmoid gating)
# for Trainium2, batch-parallel across 8 NeuronCores (one batch row per core).
#
# Per-core math (batch b):
#   q = query @ Wq + bq -> per head: q1, q2, gate (each S x 64)
#   k = key   @ Wk + bk -> per head: k1, k2
#   v = values@ Wv + bv -> per head: v (S x 64)
#   attn = softmax(q1 k1^T / 8) - lam * softmax(q2 k2^T / 8)
#   out  = GroupNorm_{8 groups over d, reduced over (S, heads, d-in-group)}(attn @ v)
#   out  = out * (1 - lambda_init) * sigmoid(gate)
#
# Layout strategy: d-major ("transposed") attention: scores are computed as
# s^T (k on partitions, q free) so the attn@v contraction runs at K=128, and
# exp row-sums come free via a ones-column appended to v (M=65).  q1/q2 (and
# k1/k2) of each head live in complementary 64-partition halves of one tile;
# the two score matmuls of a head run as K=64 matmuls on disjoint partition
# halves (tile_position row 0 / 64), so no zero-padded key copies are needed.
# Matmul inputs are bf16 (single-pass PE); accumulation, softmax
# normalization and the GroupNorm statistics stay fp32.
#
# The gated output is algebraically refactored so the whole epilogue fuses
# into the output transposes:
#   out = (a[d]*y + ball[d]) * (tanh(g/2)+1)        (a,ball fold GN+lambda)
#       = a[d]*yg + ball[d]*bgt,   yg = y*(tanh+1), bgt = tanh+1
# yg/bgt are produced during the attention phase (bf16), and the final
# scale+transpose is two accumulating PE matmuls per 128-chunk against
# diag(a) / diag(ball) "scaled identity" matrices -- no post-stats vector
# pass over the full tensor remains.

import numpy as np

B, S_FULL, H, D = 8, 1024, 8, 64
DM = H * D  # 512


def build_nc(S=1024):
    import concourse.bacc as bacc
    import concourse.bass as bass
    import concourse.tile as tile
    from concourse import mybir
    from concourse.masks import make_identity

    f32 = mybir.dt.float32
    bf16 = mybir.dt.bfloat16
    AF = mybir.ActivationFunctionType
    OP = mybir.AluOpType
    AX = mybir.AxisListType

    NJ = S // 128          # k/seq 128-tiles
    CH = min(512, S)       # fp32-out matmul chunk
    NN = max(1, S // CH)
    CNT = float(S * H * (D // H))  # groupnorm reduction count per group
    EPS = 1e-3
    INV = 0.125            # 1/sqrt(64)

    nc = bacc.Bacc(target_bir_lowering=False)
    q_d = nc.dram_tensor("query", [S, DM], f32, kind="ExternalInput")
    k_d = nc.dram_tensor("key", [S, DM], f32, kind="ExternalInput")
    v_d = nc.dram_tensor("values", [S, DM], f32, kind="ExternalInput")
    wq_d = nc.dram_tensor("Wq", [DM, 3 * H * D], f32, kind="ExternalInput")
    bq_d = nc.dram_tensor("bq", [3 * H * D], f32, kind="ExternalInput")
    wk_d = nc.dram_tensor("Wk", [DM, 2 * H * D], f32, kind="ExternalInput")
    bk_d = nc.dram_tensor("bk", [2 * H * D], f32, kind="ExternalInput")
    wv_d = nc.dram_tensor("Wv", [DM, H * D], f32, kind="ExternalInput")
    bv_d = nc.dram_tensor("bv", [H * D], f32, kind="ExternalInput")
    gamma_d = nc.dram_tensor("gamma", [D], f32, kind="ExternalInput")
    beta_d = nc.dram_tensor("beta", [D], f32, kind="ExternalInput")
    lam_d = nc.dram_tensor("lam", [1], f32, kind="ExternalInput")
    li_d = nc.dram_tensor("lambda_init", [1], f32, kind="ExternalInput")
    out_d = nc.dram_tensor("out", [S, DM], f32, kind="ExternalOutput")

    ts_ = nc.vector.tensor_scalar
    stt = nc.vector.scalar_tensor_tensor

    with tile.TileContext(nc) as tc:
        with tc.tile_pool(name="consts", bufs=1) as consts, \
             tc.tile_pool(name="persist", bufs=1) as persist:

            # bf16 identity FIRST on the gpsimd queue: the input transposes
            # depend on it, so nothing may precede it there.
            ident_b = consts.tile([128, 128], bf16, tag="ident_b", name="ident_b")
            make_identity(nc, ident_b)

            # gate-projection weight gather on SWDGE: issue right away so the
            # (slow) software DMA completes long before the gate matmuls.
            wgt = []
            for r in range(4):
                w_t = consts.tile([128, 512], bf16, tag=f"wg{r}", name=f"wg{r}")
                nc.gpsimd.dma_start(
                    out=w_t,
                    in_=wq_d[128 * r:128 * (r + 1), :].rearrange(
                        "k (h blk) -> k h blk", blk=192)[:, :, 128:192])
                wgt.append(w_t)

            # persistent projection outputs (bf16, d-major)
            # kk[h]: rows 0-63 = k1 of head h, rows 64-127 = k2.
            # qz1[h] rows 0-63 = q1 (rest 0), qz2[h] rows 64-127 = q2 (rest 0):
            # zero-padding on the q (moving) side keeps every score matmul a
            # uniform K=128/M=128 shape -- split-row-group (K=64) matmuls
            # trip the PE power governor into a sustained half-clock throttle.
            qz1 = [persist.tile([128, S], bf16, tag=f"qz1{h}", name=f"qz1{h}") for h in range(8)]
            qz2 = [persist.tile([128, S], bf16, tag=f"qz2{h}", name=f"qz2{h}") for h in range(8)]
            kk = [persist.tile([128, S], bf16, tag=f"kk{h}", name=f"kk{h}") for h in range(8)]
            # zero the pad halves on gpsimd (idle after the consts; keeps both
            # the DVE queue and the ACT epilogues unblocked)
            for h in range(8):
                nc.gpsimd.memset(qz1[h][64:128, :], 0.0)
                nc.gpsimd.memset(qz2[h][0:64, :], 0.0)
            # gate stays head-pair packed: gt[p] rows 0-63 = head 2p, 64-127 = 2p+1
            gt = [persist.tile([128, S], bf16, tag=f"gt{p}", name=f"gt{p}") for p in range(4)]
            va = [persist.tile([128, 8, 65], bf16, tag=f"va{i}", name=f"va{i}") for i in range(NJ)]
            # ypair holds yg = y*(tanh(g/2)+1); bgt holds tanh(g/2)+1 (both bf16)
            ypair = [persist.tile([128, S], bf16, tag=f"yp{p}", name=f"yp{p}") for p in range(4)]
            bgt = [persist.tile([128, S], bf16, tag=f"bgt{p}", name=f"bgt{p}") for p in range(4)]
            sumcol = persist.tile([64, 16], f32, tag="sumcol", name="sumcol")

            # ---------- phase 1: load + transpose inputs, projections ----------
            with tc.tile_pool(name="xin", bufs=3) as xin_pool, \
                 tc.tile_pool(name="xtp", bufs=1) as xtp, \
                 tc.tile_pool(name="wload", bufs=1) as wpool, \
                 tc.tile_pool(name="thp", bufs=2) as thp, \
                 tc.tile_pool(name="ps_in", bufs=1, space="PSUM") as ps_in, \
                 tc.tile_pool(name="ps_proj", bufs=4, space="PSUM") as ps_proj:

                GRP = min(4, NJ)
                # x^T tiles are shared across q/k/v (WAR deps serialize on PE
                # program order anyway; saves 16KB/partition of SBUF)
                xt = [xtp.tile([128, S], bf16, tag=f"xt{c}", name=f"xt{c}")
                      for c in range(4)]

                def transpose_input(x_dram, dst=None, mid=None, on_act=False):
                    # on_act: run the bf16 casts + PSUM drains on the scalar
                    # engine -- it is idle before the first projection
                    # epilogues, and this unblocks the PE transposes sooner
                    cp = nc.scalar.copy if on_act else nc.vector.tensor_copy
                    dst = dst if dst is not None else xt
                    tp_cur = [None] * 4
                    for i in range(NJ):
                        xs = xin_pool.tile([128, DM], f32, tag="xs", name="xs")
                        nc.sync.dma_start(out=xs, in_=x_dram[128 * i:128 * (i + 1), :])
                        xq = xin_pool.tile([128, DM], bf16, tag="xin", name="xin")
                        cp(xq, xs)
                        if i % GRP == 0:
                            for c in range(4):
                                tp_cur[c] = ps_in.tile(
                                    [128, 128 * GRP], bf16, tag=f"tp{c}", name=f"tp{c}")
                        for c in range(4):
                            nc.tensor.transpose(
                                tp_cur[c][:, 128 * (i % GRP):128 * (i % GRP + 1)],
                                xq[:, 128 * c:128 * (c + 1)], ident_b)
                        if i % GRP == GRP - 1:
                            base = 128 * GRP * (i // GRP)
                            for c in range(4):
                                cp(dst[c][:, base:base + 128 * GRP], tp_cur[c])
                        if mid is not None and i == GRP - 1:
                            mid()
                    return dst

                # fp32 weight staging (HWDGE) + DVE downcast; staging tiles are
                # shared q->k->v (WAR on the quick downcast, saves 24KB SBUF)
                wst = [wpool.tile([128, 3 * H * D], f32, tag=f"wst{r}", name=f"wst{r}")
                       for r in range(4)]
                wqf = [wpool.tile([128, 3 * H * D], bf16, tag=f"wqf{r}", name=f"wqf{r}") for r in range(4)]
                wkf = [wpool.tile([128, 2 * H * D], bf16, tag=f"wkf{r}", name=f"wkf{r}") for r in range(4)]
                wvf = [wpool.tile([128, H * D], bf16, tag=f"wvf{r}", name=f"wvf{r}") for r in range(4)]

                def stage_wq():
                    for r in range(4):
                        nc.sync.dma_start(out=wst[r], in_=wq_d[128 * r:128 * (r + 1), :])
                        nc.vector.tensor_copy(wqf[r], wst[r])

                # --- query path (its first DMAs lead the sync queue; the wq
                # staging is interleaved after the first transpose group) ---
                xtq = transpose_input(q_d, mid=stage_wq, on_act=True)
                bqp = consts.tile([128, 8], f32, tag="bqp", name="bqp")
                nc.sync.dma_start(
                    out=bqp,
                    in_=bq_d[:].rearrange("(h blk) -> blk h", blk=192)[0:128, :])
                bg = consts.tile([128, 4], f32, tag="bg", name="bg")
                bqv = bq_d[:].rearrange("(h blk) -> h blk", blk=192)
                for p in range(4):
                    nc.sync.dma_start(out=bg[:, p:p + 1],
                                      in_=bqv[2 * p:2 * p + 2, 128:192])
                for h in range(8):
                    for n in range(NN):
                        ps = ps_proj.tile([128, CH], f32, tag="proj", name="proj")
                        for r in range(4):
                            nc.tensor.matmul(
                                ps, wqf[r][:, 192 * h:192 * h + 128],
                                xtq[r][:, CH * n:CH * (n + 1)],
                                start=(r == 0), stop=(r == 3))
                        nc.scalar.activation(
                            qz1[h][0:64, CH * n:CH * (n + 1)], ps[0:64, :],
                            AF.Identity, bias=bqp[0:64, h:h + 1])
                        nc.scalar.activation(
                            qz2[h][64:128, CH * n:CH * (n + 1)], ps[64:128, :],
                            AF.Identity, bias=bqp[64:128, h:h + 1])
                for p in range(4):
                    for n in range(NN):
                        ps = ps_proj.tile([128, CH], f32, tag="proj", name="proj")
                        for r in range(4):
                            nc.tensor.matmul(
                                ps, wgt[r][:, 128 * p:128 * (p + 1)],
                                xtq[r][:, CH * n:CH * (n + 1)],
                                start=(r == 0), stop=(r == 3))
                        nc.scalar.activation(
                            gt[p][:, CH * n:CH * (n + 1)], ps, AF.Identity,
                            bias=bg[:, p:p + 1])

                # --- deferred scalar/stat constants (off the critical path) ---
                lam128 = consts.tile([128, 1], f32, tag="lam128", name="lam128")
                nc.gpsimd.dma_start(out=lam128, in_=lam_d[:].to_broadcast([128, 1]))
                li128 = consts.tile([128, 1], f32, tag="li128", name="li128")
                nc.gpsimd.dma_start(out=li128, in_=li_d[:].to_broadcast([128, 1]))
                neglam = consts.tile([128, 1], f32, tag="neglam", name="neglam")
                ts_(neglam, lam128, -1.0, None, OP.mult)
                onelam = consts.tile([128, 1], f32, tag="onelam", name="onelam")
                ts_(onelam, lam128, -1.0, 1.0, OP.mult, OP.add)   # 1 - lam
                halfli = consts.tile([128, 1], f32, tag="halfli", name="halfli")
                ts_(halfli, li128, -0.5, 0.5, OP.mult, OP.add)    # 0.5*(1-li)

                bkp = consts.tile([128, 8], f32, tag="bkp", name="bkp")
                nc.sync.dma_start(
                    out=bkp,
                    in_=bk_d[:].rearrange("(h blk) -> blk h", blk=128))

                gamma128 = consts.tile([128, 1], f32, tag="gamma128", name="gamma128")
                nc.sync.dma_start(out=gamma128[0:64, :], in_=gamma_d[:])
                nc.sync.dma_start(out=gamma128[64:128, :], in_=gamma_d[:])
                beta128 = consts.tile([128, 1], f32, tag="beta128", name="beta128")
                nc.sync.dma_start(out=beta128[0:64, :], in_=beta_d[:])
                nc.sync.dma_start(out=beta128[64:128, :], in_=beta_d[:])
                bb128 = consts.tile([128, 1], f32, tag="bb128", name="bb128")
                ts_(bb128, beta128, halfli, None, OP.mult)        # beta*0.5*(1-li)

                # v-bias columns: head-major [64,8] for the stats corrections,
                # pair-stacked [128,4] for the final affine
                bvc = consts.tile([64, 8], f32, tag="bvc", name="bvc")
                nc.sync.dma_start(
                    out=bvc, in_=bv_d[:].rearrange("(h d) -> d h", d=64))
                cc64 = consts.tile([64, 8], f32, tag="cc64", name="cc64")
                ts_(cc64, bvc, onelam[0:64, :], None, OP.mult)
                bvc128 = consts.tile([128, 4], f32, tag="bvc128", name="bvc128")
                nc.sync.dma_start(
                    out=bvc128, in_=bv_d[:].rearrange("(p k d) -> (k d) p", k=2, d=64))
                cc128 = consts.tile([128, 4], f32, tag="cc128", name="cc128")
                ts_(cc128, bvc128, onelam, None, OP.mult)
                # cc-only GroupNorm stat corrections (ready before the tail)
                csq64 = consts.tile([64, 1], f32, tag="csq64", name="csq64")
                csum64 = consts.tile([64, 1], f32, tag="csum64", name="csum64")
                ccsq = consts.tile([64, 8], f32, tag="ccsq", name="ccsq")
                nc.vector.tensor_mul(ccsq, cc64, cc64)
                nc.vector.tensor_reduce(csq64, ccsq, axis=AX.X, op=OP.add)
                nc.vector.tensor_reduce(csum64, cc64, axis=AX.X, op=OP.add)

                # group matrix for the stats matmul, duplicated across both
                # 64-row halves: ind2b[d, d'] = 1 iff d//8 == (d' mod 64)//8
                ind2b = consts.tile([64, 128], f32, tag="ind2b", name="ind2b")
                nc.gpsimd.memset(ind2b, 1.0)
                nc.gpsimd.affine_select(
                    out=ind2b, in_=ind2b, compare_op=OP.is_ge, fill=0.0,
                    base=0, pattern=[[0, 2], [-8, 8], [0, 8]], channel_multiplier=1)
                nc.gpsimd.affine_select(
                    out=ind2b, in_=ind2b, compare_op=OP.is_ge, fill=0.0,
                    base=7, pattern=[[0, 2], [8, 8], [0, 8]], channel_multiplier=-1)

                # selector for the last half's PE-broadcast of the softmax
                # normalizers: sel2[r, x] = 1 iff x//64 == r   (r in 0..1)
                # bf16 so the broadcast matmul runs at full (non-fp32) rate
                sel2 = consts.tile([2, 128], bf16, tag="sel2", name="sel2")
                nc.gpsimd.memset(sel2, 1.0)
                nc.gpsimd.affine_select(
                    out=sel2, in_=sel2, compare_op=OP.is_ge, fill=0.0,
                    base=0, pattern=[[1, 128]], channel_multiplier=-64)
                nc.gpsimd.affine_select(
                    out=sel2, in_=sel2, compare_op=OP.is_ge, fill=0.0,
                    base=63, pattern=[[-1, 128]], channel_multiplier=64)

                # --- key path ---
                xtk = transpose_input(k_d)
                for r in range(4):
                    nc.sync.dma_start(out=wst[r][:, 0:2 * H * D],
                                      in_=wk_d[128 * r:128 * (r + 1), :])
                    nc.vector.tensor_copy(wkf[r], wst[r][:, 0:2 * H * D])
                for h in range(8):
                    for n in range(NN):
                        ps = ps_proj.tile([128, CH], f32, tag="proj", name="proj")
                        for r in range(4):
                            nc.tensor.matmul(
                                ps, wkf[r][:, 128 * h:128 * (h + 1)],
                                xtk[r][:, CH * n:CH * (n + 1)],
                                start=(r == 0), stop=(r == 3))
                        nc.scalar.activation(
                            kk[h][:, CH * n:CH * (n + 1)], ps,
                            AF.Identity, bias=bkp[:, h:h + 1])

                # --- values path (q-major, interleaved into v_aug + ones) ---
                xtv = transpose_input(v_d)
                for r in range(4):
                    nc.sync.dma_start(out=wst[r][:, 0:H * D],
                                      in_=wv_d[128 * r:128 * (r + 1), :])
                    nc.vector.tensor_copy(wvf[r], wst[r][:, 0:H * D])
                for i in range(NJ):
                    ps = ps_proj.tile([128, 512], f32, tag="proj", name="proj")
                    for r in range(4):
                        nc.tensor.matmul(
                            ps, xtv[r][:, 128 * i:128 * (i + 1)], wvf[r],
                            start=(r == 0), stop=(r == 3))
                    nc.vector.tensor_copy(
                        va[i][:, :, 0:64],
                        ps.rearrange("p (h d) -> p h d", d=64))
                    nc.gpsimd.memset(va[i][:, :, 64:65], 1.0)

                # gate tanh now (ACT is free here); bgt = tanh(g/2) + 1  (bf16)
                for p in range(4):
                    th = thp.tile([128, S], f32, tag="th", name="th")
                    nc.scalar.activation(th, gt[p], AF.Tanh, scale=0.5)
                    ts_(bgt[p], th, 1.0, None, OP.add)



            # ---------- phase 2: attention per head (pairs for epilogue) ----
            with tc.tile_pool(name="ps_att", bufs=2, space="PSUM") as ps_att, \
                 tc.tile_pool(name="ps_o", bufs=2, space="PSUM") as ps_o, \
                 tc.tile_pool(name="expp", bufs=3) as expp, \
                 tc.tile_pool(name="osp", bufs=2) as osp, \
                 tc.tile_pool(name="typ", bufs=2) as typ, \
                 tc.tile_pool(name="tailp", bufs=1) as tailp, \
                 tc.tile_pool(name="oq", bufs=3) as oqp, \
                 tc.tile_pool(name="spp", bufs=2) as spp:

                ty3 = None
                for p in range(4):
                    ty = typ.tile([128, S], f32, tag="ty", name="ty")
                    for half in range(2):
                        h = 2 * p + half
                        last = (p == 3 and half == 1)
                        prow = 64 * half   # q1/k1 in rows 0-63, q2/k2 in 64-127
                        sp2 = spp.tile([2, S], f32, tag="sp2", name="sp2")
                        os_c = {}
                        # term-sequential: only one o accumulator lives at a
                        # time, so both the score tiles and the o tiles can
                        # double-buffer inside the 8-bank PSUM budget.
                        for t, qz_ in ((1, qz1), (2, qz2)):
                            o_ps = ps_o.tile([65, S], f32, tag="o", name="o")
                            for j in range(NJ):
                                s_ps = ps_att.tile([128, S], f32, tag="s", name="s")
                                for n in range(NN):
                                    nc.tensor.matmul(
                                        s_ps[:, CH * n:CH * (n + 1)],
                                        kk[h][:, 128 * j:128 * (j + 1)],
                                        qz_[h][:, CH * n:CH * (n + 1)],
                                        start=True, stop=True)
                                ex = expp.tile([128, S], bf16, tag="exp", name="exp")
                                nc.scalar.activation(ex, s_ps, AF.Exp, scale=INV)
                                for n in range(NN):
                                    nc.tensor.matmul(
                                        o_ps[:, CH * n:CH * (n + 1)],
                                        va[j][:, h, :],
                                        ex[:, CH * n:CH * (n + 1)],
                                        start=(j == 0), stop=(j == NJ - 1))
                            os_ = osp.tile([65, S], f32, tag=f"os{t}", name=f"os{t}")
                            if last:
                                # ACT is idle once the exps are done; freeing
                                # the DVE queue for the combine chain
                                nc.scalar.copy(os_, o_ps)
                            else:
                                nc.vector.tensor_copy(os_, o_ps)
                            nc.sync.dma_start(
                                out=sp2[t - 1:t, :], in_=os_[64:65, :])
                            os_c[t] = os_

                        # per-half combine: softmax-normalize, subtract the
                        # lam-weighted term, accumulate GN stats
                        rp2 = spp.tile([2, S], f32, tag="rp2", name="rp2")
                        if last:
                            # tail: fast reciprocal (~1e-3 rel err on two
                            # heads' normalizers, well inside tolerance) and
                            # a selector-matmul broadcast on the idle PE
                            nc.vector.reciprocal_approx_fast(out=rp2, in_=sp2)
                            rpb = spp.tile([2, S], bf16, tag="rpb", name="rpb")
                            nc.vector.tensor_copy(rpb, rp2)
                            bc = ps_att.tile([128, S], f32, tag="s", name="bcpe")
                            for n in range(NN):
                                nc.tensor.matmul(
                                    bc[:, CH * n:CH * (n + 1)], sel2,
                                    rpb[:, CH * n:CH * (n + 1)],
                                    start=True, stop=True)
                            bcs1, bcs2 = bc[0:64, :], bc[64:128, :]
                        else:
                            rscr = spp.tile([2, S], f32, tag="rscr", name="rscr")
                            nc.vector.reciprocal_approx_accurate(rp2, sp2, rscr)
                            bcs1 = spp.tile([64, S], f32, tag="bcs1", name="bcs1")
                            bcs2 = spp.tile([64, S], f32, tag="bcs2", name="bcs2")
                            r1 = spp.tile([1, S], f32, tag="rst1", name="rst1")
                            nc.sync.dma_start(out=r1, in_=rp2[0:1, :])
                            r2 = spp.tile([1, S], f32, tag="rst2", name="rst2")
                            nc.sync.dma_start(out=r2, in_=rp2[1:2, :])
                            nc.gpsimd.partition_broadcast(bcs1, r1[0:1, :], channels=64)
                            nc.gpsimd.partition_broadcast(bcs2, r2[0:1, :], channels=64)
                        os1, os2 = os_c[1], os_c[2]
                        nc.vector.tensor_mul(os1[0:64, :], os1[0:64, :], bcs1)
                        stt(os2[0:64, :], os2[0:64, :], neglam[0:64, :], bcs2,
                            OP.mult, OP.mult)
                        tyh = ty[prow:prow + 64, :]
                        stt(tyh, os1[0:64, :], 1.0, os2[0:64, :],
                            OP.bypass, OP.add, accum_out=sumcol[:, h:h + 1])
                        stt(os1[0:64, :], tyh, 1.0, tyh, OP.mult, OP.mult,
                            accum_out=sumcol[:, 8 + h:9 + h])
                    # yg = y * (tanh+1)  (bf16, consumed by the fused
                    # scale+transpose in phase 3).  The last pair's is
                    # deferred into the stats chain so the reductions start
                    # immediately after its ysq.
                    if p < 3:
                        nc.vector.tensor_mul(ypair[p], ty, bgt[p])
                    else:
                        ty3 = ty

                # ------- tail: stats + fused scale/transpose output -------
                # (same pool scope: ty3 must stay alive, and the stats/output
                # PSUM reuses the attention pools' banks)
                tot = tailp.tile([64, 2], f32, tag="tot", name="tot")
                nc.vector.tensor_reduce(
                    tot, sumcol.rearrange("p (t h) -> p t h", h=8),
                    axis=AX.X, op=OP.add)
                # bias-C (bv) corrections to the raw-Y stats
                csc = tailp.tile([64, 8], f32, tag="csc", name="csc")
                nc.vector.tensor_mul(csc, cc64, sumcol[:, 0:8])
                cy64 = tailp.tile([64, 1], f32, tag="cy64", name="cy64")
                nc.vector.tensor_reduce(cy64, csc, axis=AX.X, op=OP.add)
                tot2 = tailp.tile([64, 2], f32, tag="tot2", name="tot2")
                stt(tot2[:, 0:1], csum64, float(S), tot[:, 0:1], OP.mult, OP.add)
                stt(tot2[:, 1:2], cy64, 2.0, tot[:, 1:2], OP.mult, OP.add)
                stt(tot2[:, 1:2], csq64, float(S), tot2[:, 1:2], OP.mult, OP.add)

                ms_ps = ps_o.tile([128, 2], f32, tag="o", name="ms")
                nc.tensor.matmul(ms_ps, ind2b, tot2, start=True, stop=True)
                mean = tailp.tile([128, 1], f32, tag="mean", name="mean")
                ts_(mean, ms_ps[:, 0:1], 1.0 / CNT, None, OP.mult)
                e2 = tailp.tile([128, 1], f32, tag="e2", name="e2")
                ts_(e2, ms_ps[:, 1:2], 1.0 / CNT, None, OP.mult)
                nm2 = tailp.tile([128, 1], f32, tag="nm2", name="nm2")
                ts_(nm2, mean, mean, -1.0, OP.mult, OP.mult)
                veps = tailp.tile([128, 1], f32, tag="veps", name="veps")
                stt(veps, nm2, EPS, e2, OP.add, OP.add)
                # deferred last-pair gate fold: slots into the DVE queue while
                # the scalar engine loads the sqrt table set
                nc.vector.tensor_mul(ypair[3], ty3, bgt[3])
                sd = tailp.tile([128, 1], f32, tag="sd", name="sd")
                nc.scalar.activation(sd, veps, AF.Sqrt)
                rsd = tailp.tile([128, 1], f32, tag="rsd", name="rsd")
                nc.vector.reciprocal(rsd, sd)
                # one Newton step for rsqrt accuracy (ACT sqrt is loose)
                rr = tailp.tile([128, 1], f32, tag="rr", name="rr")
                nc.vector.tensor_mul(rr, rsd, rsd)
                nc.vector.tensor_mul(rr, rr, veps)
                ts_(rr, rr, -0.5, 1.5, OP.mult, OP.add)
                rstd = tailp.tile([128, 1], f32, tag="rstd", name="rstd")
                nc.vector.tensor_mul(rstd, rsd, rr)

                a128 = tailp.tile([128, 1], f32, tag="a128", name="a128")
                ts_(a128, rstd, gamma128, halfli, OP.mult, OP.mult)
                cm128 = tailp.tile([128, 4], f32, tag="cm128", name="cm128")
                ts_(cm128, cc128, mean, None, OP.subtract)
                ball = tailp.tile([128, 4], f32, tag="ball", name="ball")
                ts_(ball, cm128, a128, bb128, OP.mult, OP.add)

                # scaled-identity matrices: out = yg^T diag(a) + bgt^T diag(ball)
                da = tailp.tile([128, 128], bf16, tag="da", name="da")
                ts_(da, ident_b, a128, None, OP.mult)
                db = []
                for p in range(4):
                    d_t = tailp.tile([128, 128], bf16, tag=f"db{p}", name=f"db{p}")
                    ts_(d_t, ident_b, ball[:, p:p + 1], None, OP.mult)
                    db.append(d_t)

                for c in range(NJ):
                    tp_o = ps_att.tile([128, 512], f32, tag="s", name="tp_out")
                    for p in range(4):
                        nc.tensor.matmul(
                            tp_o[:, 128 * p:128 * (p + 1)],
                            ypair[p][:, 128 * c:128 * (c + 1)], da,
                            start=True, stop=False)
                        nc.tensor.matmul(
                            tp_o[:, 128 * p:128 * (p + 1)],
                            bgt[p][:, 128 * c:128 * (c + 1)], db[p],
                            start=False, stop=True)
                    oq = oqp.tile([128, 512], f32, tag="oq", name="oq")
                    nc.scalar.copy(oq, tp_o)
                    nc.sync.dma_start(out=out_d[128 * c:128 * (c + 1), :], in_=oq)

    nc.finalize()
    return nc


_CACHE = {}


def _get_nc():
    if "nc" not in _CACHE:
        _CACHE["nc"] = build_nc(S_FULL)
    return _CACHE["nc"]


def run(inputs, trace=False, tmpdir=None):
    from concourse.bass_utils import run_bass_kernel_spmd
    nc = _get_nc()
    arrs = {k: np.asarray(v, dtype=np.float32) for k, v in inputs.items()}
    shared = {k: np.ascontiguousarray(arrs[k]) for k in
              ("Wq", "bq", "Wk", "bk", "Wv", "bv", "gamma", "beta",
               "lam", "lambda_init")}
    in_maps = []
    for i in range(B):
        m = dict(shared)
        m["query"] = np.ascontiguousarray(arrs["query"][i])
        m["key"] = np.ascontiguousarray(arrs["key"][i])
        m["values"] = np.ascontiguousarray(arrs["values"][i])
        in_maps.append(m)
    res = run_bass_kernel_spmd(nc, in_maps, core_ids=list(range(B)),
                               trace=trace, tmpdir=tmpdir)
    out = np.stack([res.results[i]["out"] for i in range(B)], axis=0)
    return out.astype(np.float32), res


def kernel(**inputs):
    out, _ = run(inputs)
    return out


# revision 58
# speedup vs baseline: 1.1673x; 1.0013x over previous
# Differential multi-head attention (dual softmax + GroupNorm + sigmoid gating)
# for Trainium2, batch-parallel across 8 NeuronCores (one batch row per core).
#
# Per-core math (batch b):
#   q = query @ Wq + bq -> per head: q1, q2, gate (each S x 64)
#   k = key   @ Wk + bk -> per head: k1, k2
#   v = values@ Wv + bv -> per head: v (S x 64)
#   attn = softmax(q1 k1^T / 8) - lam * softmax(q2 k2^T / 8)
#   out  = GroupNorm_{8 groups over d, reduced over (S, heads, d-in-group)}(attn @ v)
#   out  = out * (1 - lambda_init) * sigmoid(gate)
#
# Layout strategy: d-major ("transposed") attention: scores are computed as
# s^T (k on partitions, q free) so the attn@v contraction runs at K=128, and
# exp row-sums come free via a ones-column appended to v (M=65).  k1/k2 of a
# head share one 128-row tile; q1/q2 are stored zero-padded (qz1 = [q1;0],
# qz2 = [0;q2]) so every score matmul keeps the uniform K=128/M=128 shape --
# split-row-group K=64 matmuls trip the PE power governor into a sustained
# half-clock throttle.  Matmul inputs are bf16 (single-pass PE);
# accumulation, softmax normalization and the GroupNorm statistics stay fp32.
#
# The gated output is algebraically refactored so the whole epilogue fuses
# into the output transposes:
#   out = (a[d]*y + ball[d]) * (tanh(g/2)+1)        (a,ball fold GN+lambda)
#       = a[d]*yg + ball[d]*bgt,   yg = y*(tanh+1), bgt = tanh+1
# yg/bgt are produced during the attention phase (bf16), and the final
# scale+transpose is two accumulating PE matmuls per 128-chunk against
# diag(a) / diag(ball) "scaled identity" matrices -- no post-stats vector
# pass over the full tensor remains.

import numpy as np

B, S_FULL, H, D = 8, 1024, 8, 64
DM = H * D  # 512


def build_nc(S=1024):
    import concourse.bacc as bacc
    import concourse.bass as bass
    import concourse.tile as tile
    from concourse import mybir
    from concourse.masks import make_identity

    f32 = mybir.dt.float32
    bf16 = mybir.dt.bfloat16
    AF = mybir.ActivationFunctionType
    OP = mybir.AluOpType
    AX = mybir.AxisListType

    NJ = S // 128          # k/seq 128-tiles
    CH = min(512, S)       # fp32-out matmul chunk
    NN = max(1, S // CH)
    CNT = float(S * H * (D // H))  # groupnorm reduction count per group
    EPS = 1e-3
    INV = 0.125            # 1/sqrt(64)

    nc = bacc.Bacc(target_bir_lowering=False)
    q_d = nc.dram_tensor("query", [S, DM], f32, kind="ExternalInput")
    k_d = nc.dram_tensor("key", [S, DM], f32, kind="ExternalInput")
    v_d = nc.dram_tensor("values", [S, DM], f32, kind="ExternalInput")
    wq_d = nc.dram_tensor("Wq", [DM, 3 * H * D], f32, kind="ExternalInput")
    bq_d = nc.dram_tensor("bq", [3 * H * D], f32, kind="ExternalInput")
    wk_d = nc.dram_tensor("Wk", [DM, 2 * H * D], f32, kind="ExternalInput")
    bk_d = nc.dram_tensor("bk", [2 * H * D], f32, kind="ExternalInput")
    wv_d = nc.dram_tensor("Wv", [DM, H * D], f32, kind="ExternalInput")
    bv_d = nc.dram_tensor("bv", [H * D], f32, kind="ExternalInput")
    gamma_d = nc.dram_tensor("gamma", [D], f32, kind="ExternalInput")
    beta_d = nc.dram_tensor("beta", [D], f32, kind="ExternalInput")
    lam_d = nc.dram_tensor("lam", [1], f32, kind="ExternalInput")
    li_d = nc.dram_tensor("lambda_init", [1], f32, kind="ExternalInput")
    out_d = nc.dram_tensor("out", [S, DM], f32, kind="ExternalOutput")

    ts_ = nc.vector.tensor_scalar
    stt = nc.vector.scalar_tensor_tensor

    with tile.TileContext(nc) as tc:
        with tc.tile_pool(name="consts", bufs=1) as consts, \
             tc.tile_pool(name="persist", bufs=1) as persist:

            # bf16 identity FIRST on the gpsimd queue: the input transposes
            # depend on it, so nothing may precede it there.
            ident_b = consts.tile([128, 128], bf16, tag="ident_b", name="ident_b")
            make_identity(nc, ident_b)

            # gate-projection weight gather on SWDGE: issue right away so the
            # (slow) software DMA completes long before the gate matmuls.
            wgt = []
            for r in range(4):
                w_t = consts.tile([128, 512], bf16, tag=f"wg{r}", name=f"wg{r}")
                nc.gpsimd.dma_start(
                    out=w_t,
                    in_=wq_d[128 * r:128 * (r + 1), :].rearrange(
                        "k (h blk) -> k h blk", blk=192)[:, :, 128:192])
                wgt.append(w_t)

            # persistent projection outputs (bf16, d-major)
            # kk[h]: rows 0-63 = k1 of head h, rows 64-127 = k2.
            # qz1[h] rows 0-63 = q1 (rest 0), qz2[h] rows 64-127 = q2 (rest 0):
            # zero-padding on the q (moving) side keeps every score matmul a
            # uniform K=128/M=128 shape -- split-row-group (K=64) matmuls
            # trip the PE power governor into a sustained half-clock throttle.
            qz1 = [persist.tile([128, S], bf16, tag=f"qz1{h}", name=f"qz1{h}") for h in range(8)]
            qz2 = [persist.tile([128, S], bf16, tag=f"qz2{h}", name=f"qz2{h}") for h in range(8)]
            kk = [persist.tile([128, S], bf16, tag=f"kk{h}", name=f"kk{h}") for h in range(8)]
            # zero the pad halves on gpsimd (idle after the consts; keeps both
            # the DVE queue and the ACT epilogues unblocked)
            for h in range(8):
                nc.gpsimd.memset(qz1[h][64:128, :], 0.0)
                nc.gpsimd.memset(qz2[h][0:64, :], 0.0)
            # gate stays head-pair packed: gt[p] rows 0-63 = head 2p, 64-127 = 2p+1
            gt = [persist.tile([128, S], bf16, tag=f"gt{p}", name=f"gt{p}") for p in range(4)]
            va = [persist.tile([128, 8, 65], bf16, tag=f"va{i}", name=f"va{i}") for i in range(NJ)]
            # ypair holds yg = y*(tanh(g/2)+1); bgt holds tanh(g/2)+1 (both bf16)
            ypair = [persist.tile([128, S], bf16, tag=f"yp{p}", name=f"yp{p}") for p in range(4)]
            bgt = [persist.tile([128, S], bf16, tag=f"bgt{p}", name=f"bgt{p}") for p in range(4)]
            sumcol = persist.tile([64, 16], f32, tag="sumcol", name="sumcol")

            # ---------- phase 1: load + transpose inputs, projections ----------
            with tc.tile_pool(name="xin", bufs=3) as xin_pool, \
                 tc.tile_pool(name="xtp", bufs=1) as xtp, \
                 tc.tile_pool(name="wload", bufs=1) as wpool, \
                 tc.tile_pool(name="thp", bufs=2) as thp, \
                 tc.tile_pool(name="ps_in", bufs=1, space="PSUM") as ps_in, \
                 tc.tile_pool(name="ps_proj", bufs=4, space="PSUM") as ps_proj:

                GRP = min(4, NJ)
                # x^T tiles are shared across q/k/v (WAR deps serialize on PE
                # program order anyway; saves 16KB/partition of SBUF)
                xt = [xtp.tile([128, S], bf16, tag=f"xt{c}", name=f"xt{c}")
                      for c in range(4)]

                def transpose_input(x_dram, dst=None, mid=None, on_act=False):
                    # on_act: run the bf16 casts + PSUM drains on the scalar
                    # engine -- it is idle before the first projection
                    # epilogues, and this unblocks the PE transposes sooner
                    cp = nc.scalar.copy if on_act else nc.vector.tensor_copy
                    dst = dst if dst is not None else xt
                    tp_cur = [None] * 4
                    for i in range(NJ):
                        xs = xin_pool.tile([128, DM], f32, tag="xs", name="xs")
                        nc.sync.dma_start(out=xs, in_=x_dram[128 * i:128 * (i + 1), :])
                        xq = xin_pool.tile([128, DM], bf16, tag="xin", name="xin")
                        cp(xq, xs)
                        if i % GRP == 0:
                            for c in range(4):
                                tp_cur[c] = ps_in.tile(
                                    [128, 128 * GRP], bf16, tag=f"tp{c}", name=f"tp{c}")
                        for c in range(4):
                            nc.tensor.transpose(
                                tp_cur[c][:, 128 * (i % GRP):128 * (i % GRP + 1)],
                                xq[:, 128 * c:128 * (c + 1)], ident_b)
                        if i % GRP == GRP - 1:
                            base = 128 * GRP * (i // GRP)
                            for c in range(4):
                                cp(dst[c][:, base:base + 128 * GRP], tp_cur[c])
                        if mid is not None and i == GRP - 1:
                            mid()
                    return dst

                # fp32 weight staging (HWDGE) + DVE downcast; staging tiles are
                # shared q->k->v (WAR on the quick downcast, saves 24KB SBUF)
                wst = [wpool.tile([128, 3 * H * D], f32, tag=f"wst{r}", name=f"wst{r}")
                       for r in range(4)]
                wqf = [wpool.tile([128, 3 * H * D], bf16, tag=f"wqf{r}", name=f"wqf{r}") for r in range(4)]
                wkf = [wpool.tile([128, 2 * H * D], bf16, tag=f"wkf{r}", name=f"wkf{r}") for r in range(4)]
                wvf = [wpool.tile([128, H * D], bf16, tag=f"wvf{r}", name=f"wvf{r}") for r in range(4)]

                def stage_wq():
                    for r in range(4):
                        nc.sync.dma_start(out=wst[r], in_=wq_d[128 * r:128 * (r + 1), :])
                        nc.vector.tensor_copy(wqf[r], wst[r])

                # --- query path (its first DMAs lead the sync queue; the wq
                # staging is interleaved after the first transpose group) ---
                xtq = transpose_input(q_d, mid=stage_wq, on_act=True)
                bqp = consts.tile([128, 8], f32, tag="bqp", name="bqp")
                nc.sync.dma_start(
                    out=bqp,
                    in_=bq_d[:].rearrange("(h blk) -> blk h", blk=192)[0:128, :])
                bg = consts.tile([128, 4], f32, tag="bg", name="bg")
                bqv = bq_d[:].rearrange("(h blk) -> h blk", blk=192)
                for p in range(4):
                    nc.sync.dma_start(out=bg[:, p:p + 1],
                                      in_=bqv[2 * p:2 * p + 2, 128:192])
                for h in range(8):
                    for n in range(NN):
                        ps = ps_proj.tile([128, CH], f32, tag="proj", name="proj")
                        for r in range(4):
                            nc.tensor.matmul(
                                ps, wqf[r][:, 192 * h:192 * h + 128],
                                xtq[r][:, CH * n:CH * (n + 1)],
                                start=(r == 0), stop=(r == 3))
                        nc.scalar.activation(
                            qz1[h][0:64, CH * n:CH * (n + 1)], ps[0:64, :],
                            AF.Identity, bias=bqp[0:64, h:h + 1])
                        nc.scalar.activation(
                            qz2[h][64:128, CH * n:CH * (n + 1)], ps[64:128, :],
                            AF.Identity, bias=bqp[64:128, h:h + 1])
                for p in range(4):
                    for n in range(NN):
                        ps = ps_proj.tile([128, CH], f32, tag="proj", name="proj")
                        for r in range(4):
                            nc.tensor.matmul(
                                ps, wgt[r][:, 128 * p:128 * (p + 1)],
                                xtq[r][:, CH * n:CH * (n + 1)],
                                start=(r == 0), stop=(r == 3))
                        nc.scalar.activation(
                            gt[p][:, CH * n:CH * (n + 1)], ps, AF.Identity,
                            bias=bg[:, p:p + 1])

                # --- deferred scalar/stat constants (off the critical path) ---
                lam128 = consts.tile([128, 1], f32, tag="lam128", name="lam128")
                nc.gpsimd.dma_start(out=lam128, in_=lam_d[:].to_broadcast([128, 1]))
                li128 = consts.tile([128, 1], f32, tag="li128", name="li128")
                nc.gpsimd.dma_start(out=li128, in_=li_d[:].to_broadcast([128, 1]))
                neglam = consts.tile([128, 1], f32, tag="neglam", name="neglam")
                ts_(neglam, lam128, -1.0, None, OP.mult)
                onelam = consts.tile([128, 1], f32, tag="onelam", name="onelam")
                ts_(onelam, lam128, -1.0, 1.0, OP.mult, OP.add)   # 1 - lam
                halfli = consts.tile([128, 1], f32, tag="halfli", name="halfli")
                ts_(halfli, li128, -0.5, 0.5, OP.mult, OP.add)    # 0.5*(1-li)

                bkp = consts.tile([128, 8], f32, tag="bkp", name="bkp")
                nc.sync.dma_start(
                    out=bkp,
                    in_=bk_d[:].rearrange("(h blk) -> blk h", blk=128))

                gamma128 = consts.tile([128, 1], f32, tag="gamma128", name="gamma128")
                nc.sync.dma_start(out=gamma128[0:64, :], in_=gamma_d[:])
                nc.sync.dma_start(out=gamma128[64:128, :], in_=gamma_d[:])
                beta128 = consts.tile([128, 1], f32, tag="beta128", name="beta128")
                nc.sync.dma_start(out=beta128[0:64, :], in_=beta_d[:])
                nc.sync.dma_start(out=beta128[64:128, :], in_=beta_d[:])
                bb128 = consts.tile([128, 1], f32, tag="bb128", name="bb128")
                ts_(bb128, beta128, halfli, None, OP.mult)        # beta*0.5*(1-li)

                # v-bias columns: head-major [64,8] for the stats corrections,
                # pair-stacked [128,4] for the final affine
                bvc = consts.tile([64, 8], f32, tag="bvc", name="bvc")
                nc.sync.dma_start(
                    out=bvc, in_=bv_d[:].rearrange("(h d) -> d h", d=64))
                cc64 = consts.tile([64, 8], f32, tag="cc64", name="cc64")
                ts_(cc64, bvc, onelam[0:64, :], None, OP.mult)
                bvc128 = consts.tile([128, 4], f32, tag="bvc128", name="bvc128")
                nc.sync.dma_start(
                    out=bvc128, in_=bv_d[:].rearrange("(p k d) -> (k d) p", k=2, d=64))
                cc128 = consts.tile([128, 4], f32, tag="cc128", name="cc128")
                ts_(cc128, bvc128, onelam, None, OP.mult)
                # cc-only GroupNorm stat corrections (ready before the tail)
                csq64 = consts.tile([64, 1], f32, tag="csq64", name="csq64")
                csum64 = consts.tile([64, 1], f32, tag="csum64", name="csum64")
                ccsq = consts.tile([64, 8], f32, tag="ccsq", name="ccsq")
                nc.vector.tensor_mul(ccsq, cc64, cc64)
                nc.vector.tensor_reduce(csq64, ccsq, axis=AX.X, op=OP.add)
                nc.vector.tensor_reduce(csum64, cc64, axis=AX.X, op=OP.add)

                # group matrix for the stats matmul, duplicated across both
                # 64-row halves: ind2b[d, d'] = 1 iff d//8 == (d' mod 64)//8
                ind2b = consts.tile([64, 128], f32, tag="ind2b", name="ind2b")
                nc.gpsimd.memset(ind2b, 1.0)
                nc.gpsimd.affine_select(
                    out=ind2b, in_=ind2b, compare_op=OP.is_ge, fill=0.0,
                    base=0, pattern=[[0, 2], [-8, 8], [0, 8]], channel_multiplier=1)
                nc.gpsimd.affine_select(
                    out=ind2b, in_=ind2b, compare_op=OP.is_ge, fill=0.0,
                    base=7, pattern=[[0, 2], [8, 8], [0, 8]], channel_multiplier=-1)

                # selector for the last half's PE-broadcast of the softmax
                # normalizers: sel2[r, x] = 1 iff x//64 == r   (r in 0..1)
                # bf16 so the broadcast matmul runs at full (non-fp32) rate
                sel2 = consts.tile([2, 128], bf16, tag="sel2", name="sel2")
                nc.gpsimd.memset(sel2, 1.0)
                nc.gpsimd.affine_select(
                    out=sel2, in_=sel2, compare_op=OP.is_ge, fill=0.0,
                    base=0, pattern=[[1, 128]], channel_multiplier=-64)
                nc.gpsimd.affine_select(
                    out=sel2, in_=sel2, compare_op=OP.is_ge, fill=0.0,
                    base=63, pattern=[[-1, 128]], channel_multiplier=64)

                # --- key path ---
                xtk = transpose_input(k_d)
                for r in range(4):
                    nc.sync.dma_start(out=wst[r][:, 0:2 * H * D],
                                      in_=wk_d[128 * r:128 * (r + 1), :])
                    nc.vector.tensor_copy(wkf[r], wst[r][:, 0:2 * H * D])
                for h in range(8):
                    for n in range(NN):
                        ps = ps_proj.tile([128, CH], f32, tag="proj", name="proj")
                        for r in range(4):
                            nc.tensor.matmul(
                                ps, wkf[r][:, 128 * h:128 * (h + 1)],
                                xtk[r][:, CH * n:CH * (n + 1)],
                                start=(r == 0), stop=(r == 3))
                        nc.scalar.activation(
                            kk[h][:, CH * n:CH * (n + 1)], ps,
                            AF.Identity, bias=bkp[:, h:h + 1])

                # --- values path (q-major, interleaved into v_aug + ones) ---
                xtv = transpose_input(v_d)
                for r in range(4):
                    nc.sync.dma_start(out=wst[r][:, 0:H * D],
                                      in_=wv_d[128 * r:128 * (r + 1), :])
                    nc.vector.tensor_copy(wvf[r], wst[r][:, 0:H * D])
                for i in range(NJ):
                    ps = ps_proj.tile([128, 512], f32, tag="proj", name="proj")
                    for r in range(4):
                        nc.tensor.matmul(
                            ps, xtv[r][:, 128 * i:128 * (i + 1)], wvf[r],
                            start=(r == 0), stop=(r == 3))
                    nc.vector.tensor_copy(
                        va[i][:, :, 0:64],
                        ps.rearrange("p (h d) -> p h d", d=64))
                    nc.gpsimd.memset(va[i][:, :, 64:65], 1.0)

                # gate tanh now (ACT is free here); bgt = tanh(g/2) + 1  (bf16)
                for p in range(4):
                    th = thp.tile([128, S], f32, tag="th", name="th")
                    nc.scalar.activation(th, gt[p], AF.Tanh, scale=0.5)
                    ts_(bgt[p], th, 1.0, None, OP.add)



            # ---------- phase 2: attention per head (pairs for epilogue) ----
            with tc.tile_pool(name="ps_att", bufs=2, space="PSUM") as ps_att, \
                 tc.tile_pool(name="ps_o", bufs=2, space="PSUM") as ps_o, \
                 tc.tile_pool(name="expp", bufs=3) as expp, \
                 tc.tile_pool(name="osp", bufs=2) as osp, \
                 tc.tile_pool(name="typ", bufs=2) as typ, \
                 tc.tile_pool(name="tailp", bufs=1) as tailp, \
                 tc.tile_pool(name="oq", bufs=3) as oqp, \
                 tc.tile_pool(name="spp", bufs=2) as spp:

                ty3 = None
                for p in range(4):
                    ty = typ.tile([128, S], f32, tag="ty", name="ty")
                    for half in range(2):
                        h = 2 * p + half
                        last = (p == 3 and half == 1)
                        prow = 64 * half   # q1/k1 in rows 0-63, q2/k2 in 64-127
                        sp2 = spp.tile([2, S], f32, tag="sp2", name="sp2")
                        os_c = {}
                        # term-sequential: only one o accumulator lives at a
                        # time, so both the score tiles and the o tiles can
                        # double-buffer inside the 8-bank PSUM budget.
                        for t, qz_ in ((1, qz1), (2, qz2)):
                            o_ps = ps_o.tile([65, S], f32, tag="o", name="o")
                            for j in range(NJ):
                                s_ps = ps_att.tile([128, S], f32, tag="s", name="s")
                                for n in range(NN):
                                    nc.tensor.matmul(
                                        s_ps[:, CH * n:CH * (n + 1)],
                                        kk[h][:, 128 * j:128 * (j + 1)],
                                        qz_[h][:, CH * n:CH * (n + 1)],
                                        start=True, stop=True)
                                ex = expp.tile([128, S], bf16, tag="exp", name="exp")
                                nc.scalar.activation(ex, s_ps, AF.Exp, scale=INV)
                                for n in range(NN):
                                    nc.tensor.matmul(
                                        o_ps[:, CH * n:CH * (n + 1)],
                                        va[j][:, h, :],
                                        ex[:, CH * n:CH * (n + 1)],
                                        start=(j == 0), stop=(j == NJ - 1))
                            os_ = osp.tile([65, S], f32, tag=f"os{t}", name=f"os{t}")
                            if last:
                                # ACT is idle once the exps are done; freeing
                                # the DVE queue for the combine chain
                                nc.scalar.copy(os_, o_ps)
                            else:
                                nc.vector.tensor_copy(os_, o_ps)
                            nc.sync.dma_start(
                                out=sp2[t - 1:t, :], in_=os_[64:65, :])
                            os_c[t] = os_

                        # per-half combine: softmax-normalize, subtract the
                        # lam-weighted term, accumulate GN stats
                        rp2 = spp.tile([2, S], f32, tag="rp2", name="rp2")
                        if last:
                            # tail: fast reciprocal (~1e-3 rel err on two
                            # heads' normalizers, well inside tolerance) and
                            # a selector-matmul broadcast on the idle PE
                            nc.vector.reciprocal_approx_fast(out=rp2, in_=sp2)
                            rpb = spp.tile([2, S], bf16, tag="rpb", name="rpb")
                            nc.vector.tensor_copy(rpb, rp2)
                            bc = ps_att.tile([128, S], f32, tag="s", name="bcpe")
                            for n in range(NN):
                                nc.tensor.matmul(
                                    bc[:, CH * n:CH * (n + 1)], sel2,
                                    rpb[:, CH * n:CH * (n + 1)],
                                    start=True, stop=True)
                            bcs1, bcs2 = bc[0:64, :], bc[64:128, :]
                        else:
                            rscr = spp.tile([2, S], f32, tag="rscr", name="rscr")
                            nc.vector.reciprocal_approx_accurate(rp2, sp2, rscr)
                            bcs1 = spp.tile([64, S], f32, tag="bcs1", name="bcs1")
                            bcs2 = spp.tile([64, S], f32, tag="bcs2", name="bcs2")
                            r1 = spp.tile([1, S], f32, tag="rst1", name="rst1")
                            nc.sync.dma_start(out=r1, in_=rp2[0:1, :])
                            r2 = spp.tile([1, S], f32, tag="rst2", name="rst2")
                            nc.sync.dma_start(out=r2, in_=rp2[1:2, :])
                            nc.gpsimd.partition_broadcast(bcs1, r1[0:1, :], channels=64)
                            nc.gpsimd.partition_broadcast(bcs2, r2[0:1, :], channels=64)
                        os1, os2 = os_c[1], os_c[2]
                        nc.vector.tensor_mul(os1[0:64, :], os1[0:64, :], bcs1)
                        stt(os2[0:64, :], os2[0:64, :], neglam[0:64, :], bcs2,
                            OP.mult, OP.mult)
                        tyh = ty[prow:prow + 64, :]
                        stt(tyh, os1[0:64, :], 1.0, os2[0:64, :],
                            OP.bypass, OP.add, accum_out=sumcol[:, h:h + 1])
                        stt(os1[0:64, :], tyh, 1.0, tyh, OP.mult, OP.mult,
                            accum_out=sumcol[:, 8 + h:9 + h])
                    # yg = y * (tanh+1)  (bf16, consumed by the fused
                    # scale+transpose in phase 3).  The last pair's is
                    # deferred into the stats chain so the reductions start
                    # immediately after its ysq.
                    if p < 3:
                        nc.vector.tensor_mul(ypair[p], ty, bgt[p])
                    else:
                        ty3 = ty

                # ------- tail: stats + fused scale/transpose output -------
                # (same pool scope: ty3 must stay alive, and the stats/output
                # PSUM reuses the attention pools' banks)
                tot = tailp.tile([64, 2], f32, tag="tot", name="tot")
                nc.vector.tensor_reduce(
                    tot, sumcol.rearrange("p (t h) -> p t h", h=8),
                    axis=AX.X, op=OP.add)
                # bias-C (bv) corrections to the raw-Y stats
                csc = tailp.tile([64, 8], f32, tag="csc", name="csc")
                nc.vector.tensor_mul(csc, cc64, sumcol[:, 0:8])
                cy64 = tailp.tile([64, 1], f32, tag="cy64", name="cy64")
                nc.vector.tensor_reduce(cy64, csc, axis=AX.X, op=OP.add)
                tot2 = tailp.tile([64, 2], f32, tag="tot2", name="tot2")
                stt(tot2[:, 0:1], csum64, float(S), tot[:, 0:1], OP.mult, OP.add)
                stt(tot2[:, 1:2], cy64, 2.0, tot[:, 1:2], OP.mult, OP.add)
                stt(tot2[:, 1:2], csq64, float(S), tot2[:, 1:2], OP.mult, OP.add)

                ms_ps = ps_o.tile([128, 2], f32, tag="o", name="ms")
                nc.tensor.matmul(ms_ps, ind2b, tot2, start=True, stop=True)
                mean = tailp.tile([128, 1], f32, tag="mean", name="mean")
                ts_(mean, ms_ps[:, 0:1], 1.0 / CNT, None, OP.mult)
                e2 = tailp.tile([128, 1], f32, tag="e2", name="e2")
                ts_(e2, ms_ps[:, 1:2], 1.0 / CNT, None, OP.mult)
                nm2 = tailp.tile([128, 1], f32, tag="nm2", name="nm2")
                ts_(nm2, mean, mean, -1.0, OP.mult, OP.mult)
                veps = tailp.tile([128, 1], f32, tag="veps", name="veps")
                stt(veps, nm2, EPS, e2, OP.add, OP.add)
                # deferred last-pair gate fold: slots into the DVE queue while
                # the scalar engine loads the sqrt table set
                nc.vector.tensor_mul(ypair[3], ty3, bgt[3])
                sd = tailp.tile([128, 1], f32, tag="sd", name="sd")
                nc.scalar.activation(sd, veps, AF.Sqrt)
                rsd = tailp.tile([128, 1], f32, tag="rsd", name="rsd")
                nc.vector.reciprocal(rsd, sd)
                # one Newton step for rsqrt accuracy (ACT sqrt is loose)
                rr = tailp.tile([128, 1], f32, tag="rr", name="rr")
                nc.vector.tensor_mul(rr, rsd, rsd)
                nc.vector.tensor_mul(rr, rr, veps)
                ts_(rr, rr, -0.5, 1.5, OP.mult, OP.add)
                rstd = tailp.tile([128, 1], f32, tag="rstd", name="rstd")
                nc.vector.tensor_mul(rstd, rsd, rr)

                a128 = tailp.tile([128, 1], f32, tag="a128", name="a128")
                ts_(a128, rstd, gamma128, halfli, OP.mult, OP.mult)
                cm128 = tailp.tile([128, 4], f32, tag="cm128", name="cm128")
                ts_(cm128, cc128, mean, None, OP.subtract)
                ball = tailp.tile([128, 4], f32, tag="ball", name="ball")
                ts_(ball, cm128, a128, bb128, OP.mult, OP.add)

                # scaled-identity matrices: out = yg^T diag(a) + bgt^T diag(ball)
                da = tailp.tile([128, 128], bf16, tag="da", name="da")
                ts_(da, ident_b, a128, None, OP.mult)
                db = []
                for p in range(4):
                    d_t = tailp.tile([128, 128], bf16, tag=f"db{p}", name=f"db{p}")
                    ts_(d_t, ident_b, ball[:, p:p + 1], None, OP.mult)
                    db.append(d_t)

                for c in range(NJ):
                    tp_o = ps_att.tile([128, 512], f32, tag="s", name="tp_out")
                    for p in range(4):
                        nc.tensor.matmul(
                            tp_o[:, 128 * p:128 * (p + 1)],
                            ypair[p][:, 128 * c:128 * (c + 1)], da,
                            start=True, stop=False)
                        nc.tensor.matmul(
                            tp_o[:, 128 * p:128 * (p + 1)],
                            bgt[p][:, 128 * c:128 * (c + 1)], db[p],
                            start=False, stop=True)
                    oq = oqp.tile([128, 512], f32, tag="oq", name="oq")
                    nc.scalar.copy(oq, tp_o)
                    nc.sync.dma_start(out=out_d[128 * c:128 * (c + 1), :], in_=oq)

    nc.finalize()
    return nc


_CACHE = {}


def _get_nc():
    if "nc" not in _CACHE:
        _CACHE["nc"] = build_nc(S_FULL)
    return _CACHE["nc"]


def run(inputs, trace=False, tmpdir=None):
    from concourse.bass_utils import run_bass_kernel_spmd
    nc = _get_nc()
    arrs = {k: np.asarray(v, dtype=np.float32) for k, v in inputs.items()}
    shared = {k: np.ascontiguousarray(arrs[k]) for k in
              ("Wq", "bq", "Wk", "bk", "Wv", "bv", "gamma", "beta",
               "lam", "lambda_init")}
    in_maps = []
    for i in range(B):
        m = dict(shared)
        m["query"] = np.ascontiguousarray(arrs["query"][i])
        m["key"] = np.ascontiguousarray(arrs["key"][i])
        m["values"] = np.ascontiguousarray(arrs["values"][i])
        in_maps.append(m)
    res = run_bass_kernel_spmd(nc, in_maps, core_ids=list(range(B)),
                               trace=trace, tmpdir=tmpdir)
    out = np.stack([res.results[i]["out"] for i in range(B)], axis=0)
    return out.astype(np.float32), res


def kernel(**inputs):
    out, _ = run(inputs)
    return out


# revision 59
# speedup vs baseline: 1.1801x; 1.0110x over previous
# Differential multi-head attention (dual softmax + GroupNorm + sigmoid gating)
# for Trainium2, batch-parallel across 8 NeuronCores (one batch row per core).
#
# Per-core math (batch b):
#   q = query @ Wq + bq -> per head: q1, q2, gate (each S x 64)
#   k = key   @ Wk + bk -> per head: k1, k2
#   v = values@ Wv + bv -> per head: v (S x 64)
#   attn = softmax(q1 k1^T / 8) - lam * softmax(q2 k2^T / 8)
#   out  = GroupNorm_{8 groups over d, reduced over (S, heads, d-in-group)}(attn @ v)
#   out  = out * (1 - lambda_init) * sigmoid(gate)
#
# Layout strategy: d-major ("transposed") attention: scores are computed as
# s^T (k on partitions, q free) so the attn@v contraction runs at K=128, and
# exp row-sums come free via a ones-column appended to v (M=65).  k1/k2 of a
# head share one 128-row tile; q1/q2 are stored zero-padded (qz1 = [q1;0],
# qz2 = [0;q2]) so every score matmul keeps the uniform K=128/M=128 shape --
# split-row-group K=64 matmuls trip the PE power governor into a sustained
# half-clock throttle.  Matmul inputs are bf16 (single-pass PE);
# accumulation, softmax normalization and the GroupNorm statistics stay fp32.
#
# The gated output is algebraically refactored so the whole epilogue fuses
# into the output transposes:
#   out = (a[d]*y + ball[d]) * (tanh(g/2)+1)        (a,ball fold GN+lambda)
#       = a[d]*yg + ball[d]*bgt,   yg = y*(tanh+1), bgt = tanh+1
# yg/bgt are produced during the attention phase (bf16), and the final
# scale+transpose is two accumulating PE matmuls per 128-chunk against
# diag(a) / diag(ball) "scaled identity" matrices -- no post-stats vector
# pass over the full tensor remains.

import numpy as np

B, S_FULL, H, D = 8, 1024, 8, 64
DM = H * D  # 512


def build_nc(S=1024):
    import concourse.bacc as bacc
    import concourse.bass as bass
    import concourse.tile as tile
    from concourse import mybir
    from concourse.masks import make_identity

    f32 = mybir.dt.float32
    bf16 = mybir.dt.bfloat16
    AF = mybir.ActivationFunctionType
    OP = mybir.AluOpType
    AX = mybir.AxisListType

    NJ = S // 128          # k/seq 128-tiles
    CH = min(512, S)       # fp32-out matmul chunk
    NN = max(1, S // CH)
    CNT = float(S * H * (D // H))  # groupnorm reduction count per group
    EPS = 1e-3
    INV = 0.125            # 1/sqrt(64)

    nc = bacc.Bacc(target_bir_lowering=False)
    q_d = nc.dram_tensor("query", [S, DM], f32, kind="ExternalInput")
    k_d = nc.dram_tensor("key", [S, DM], f32, kind="ExternalInput")
    v_d = nc.dram_tensor("values", [S, DM], f32, kind="ExternalInput")
    wq_d = nc.dram_tensor("Wq", [DM, 3 * H * D], f32, kind="ExternalInput")
    bq_d = nc.dram_tensor("bq", [3 * H * D], f32, kind="ExternalInput")
    wk_d = nc.dram_tensor("Wk", [DM, 2 * H * D], f32, kind="ExternalInput")
    bk_d = nc.dram_tensor("bk", [2 * H * D], f32, kind="ExternalInput")
    wv_d = nc.dram_tensor("Wv", [DM, H * D], f32, kind="ExternalInput")
    bv_d = nc.dram_tensor("bv", [H * D], f32, kind="ExternalInput")
    gamma_d = nc.dram_tensor("gamma", [D], f32, kind="ExternalInput")
    beta_d = nc.dram_tensor("beta", [D], f32, kind="ExternalInput")
    lam_d = nc.dram_tensor("lam", [1], f32, kind="ExternalInput")
    li_d = nc.dram_tensor("lambda_init", [1], f32, kind="ExternalInput")
    out_d = nc.dram_tensor("out", [S, DM], f32, kind="ExternalOutput")

    ts_ = nc.vector.tensor_scalar
    stt = nc.vector.scalar_tensor_tensor

    with tile.TileContext(nc) as tc:
        with tc.tile_pool(name="consts", bufs=1) as consts, \
             tc.tile_pool(name="persist", bufs=1) as persist:

            # bf16 identity FIRST on the gpsimd queue: the input transposes
            # depend on it, so nothing may precede it there.
            ident_b = consts.tile([128, 128], bf16, tag="ident_b", name="ident_b")
            make_identity(nc, ident_b)

            # gate-projection weight gather on SWDGE: issue right away so the
            # (slow) software DMA completes long before the gate matmuls.
            wgt = []
            for r in range(4):
                w_t = consts.tile([128, 512], bf16, tag=f"wg{r}", name=f"wg{r}")
                nc.gpsimd.dma_start(
                    out=w_t,
                    in_=wq_d[128 * r:128 * (r + 1), :].rearrange(
                        "k (h blk) -> k h blk", blk=192)[:, :, 128:192])
                wgt.append(w_t)

            # persistent projection outputs (bf16, d-major)
            # kk[h]: rows 0-63 = k1 of head h, rows 64-127 = k2.
            # qz1[h] rows 0-63 = q1 (rest 0), qz2[h] rows 64-127 = q2 (rest 0):
            # zero-padding on the q (moving) side keeps every score matmul a
            # uniform K=128/M=128 shape -- split-row-group (K=64) matmuls
            # trip the PE power governor into a sustained half-clock throttle.
            qz1 = [persist.tile([128, S], bf16, tag=f"qz1{h}", name=f"qz1{h}") for h in range(8)]
            qz2 = [persist.tile([128, S], bf16, tag=f"qz2{h}", name=f"qz2{h}") for h in range(8)]
            kk = [persist.tile([128, S], bf16, tag=f"kk{h}", name=f"kk{h}") for h in range(8)]
            # zero the pad halves on gpsimd (idle after the consts; keeps both
            # the DVE queue and the ACT epilogues unblocked)
            for h in range(8):
                nc.gpsimd.memset(qz1[h][64:128, :], 0.0)
                nc.gpsimd.memset(qz2[h][0:64, :], 0.0)
            # gate stays head-pair packed: gt[p] rows 0-63 = head 2p, 64-127 = 2p+1
            gt = [persist.tile([128, S], bf16, tag=f"gt{p}", name=f"gt{p}") for p in range(4)]
            va = [persist.tile([128, 8, 65], bf16, tag=f"va{i}", name=f"va{i}") for i in range(NJ)]
            # ypair holds yg = y*(tanh(g/2)+1); bgt holds tanh(g/2)+1 (both bf16)
            ypair = [persist.tile([128, S], bf16, tag=f"yp{p}", name=f"yp{p}") for p in range(4)]
            bgt = [persist.tile([128, S], bf16, tag=f"bgt{p}", name=f"bgt{p}") for p in range(4)]
            sumcol = persist.tile([64, 16], f32, tag="sumcol", name="sumcol")

            # ---------- phase 1: load + transpose inputs, projections ----------
            with tc.tile_pool(name="xin", bufs=3) as xin_pool, \
                 tc.tile_pool(name="xtp", bufs=1) as xtp, \
                 tc.tile_pool(name="wload", bufs=1) as wpool, \
                 tc.tile_pool(name="thp", bufs=2) as thp, \
                 tc.tile_pool(name="ps_in", bufs=1, space="PSUM") as ps_in, \
                 tc.tile_pool(name="ps_proj", bufs=4, space="PSUM") as ps_proj:

                GRP = min(4, NJ)
                # x^T tiles are shared across q/k/v (WAR deps serialize on PE
                # program order anyway; saves 16KB/partition of SBUF)
                xt = [xtp.tile([128, S], bf16, tag=f"xt{c}", name=f"xt{c}")
                      for c in range(4)]

                def transpose_input(x_dram, dst=None, mid=None, on_act=False):
                    # on_act: run the bf16 casts + PSUM drains on the scalar
                    # engine -- it is idle before the first projection
                    # epilogues, and this unblocks the PE transposes sooner
                    cp = nc.scalar.copy if on_act else nc.vector.tensor_copy
                    dst = dst if dst is not None else xt
                    tp_cur = [None] * 4
                    for i in range(NJ):
                        xs = xin_pool.tile([128, DM], f32, tag="xs", name="xs")
                        nc.sync.dma_start(out=xs, in_=x_dram[128 * i:128 * (i + 1), :])
                        xq = xin_pool.tile([128, DM], bf16, tag="xin", name="xin")
                        cp(xq, xs)
                        if i % GRP == 0:
                            for c in range(4):
                                tp_cur[c] = ps_in.tile(
                                    [128, 128 * GRP], bf16, tag=f"tp{c}", name=f"tp{c}")
                        for c in range(4):
                            nc.tensor.transpose(
                                tp_cur[c][:, 128 * (i % GRP):128 * (i % GRP + 1)],
                                xq[:, 128 * c:128 * (c + 1)], ident_b)
                        if i % GRP == GRP - 1:
                            base = 128 * GRP * (i // GRP)
                            for c in range(4):
                                cp(dst[c][:, base:base + 128 * GRP], tp_cur[c])
                        if mid is not None and i == GRP - 1:
                            mid()
                    return dst

                # fp32 weight staging (HWDGE) + DVE downcast; staging tiles are
                # shared q->k->v (WAR on the quick downcast, saves 24KB SBUF)
                wst = [wpool.tile([128, 3 * H * D], f32, tag=f"wst{r}", name=f"wst{r}")
                       for r in range(4)]
                wqf = [wpool.tile([128, 3 * H * D], bf16, tag=f"wqf{r}", name=f"wqf{r}") for r in range(4)]
                wkf = [wpool.tile([128, 2 * H * D], bf16, tag=f"wkf{r}", name=f"wkf{r}") for r in range(4)]
                wvf = [wpool.tile([128, H * D], bf16, tag=f"wvf{r}", name=f"wvf{r}") for r in range(4)]

                def stage_wq():
                    for r in range(4):
                        nc.sync.dma_start(out=wst[r], in_=wq_d[128 * r:128 * (r + 1), :])
                        nc.vector.tensor_copy(wqf[r], wst[r])

                # --- query path (its first DMAs lead the sync queue; the wq
                # staging is interleaved after the first transpose group) ---
                xtq = transpose_input(q_d, mid=stage_wq, on_act=True)
                bqp = consts.tile([128, 8], f32, tag="bqp", name="bqp")
                nc.sync.dma_start(
                    out=bqp,
                    in_=bq_d[:].rearrange("(h blk) -> blk h", blk=192)[0:128, :])
                bg = consts.tile([128, 4], f32, tag="bg", name="bg")
                bqv = bq_d[:].rearrange("(h blk) -> h blk", blk=192)
                for p in range(4):
                    nc.sync.dma_start(out=bg[:, p:p + 1],
                                      in_=bqv[2 * p:2 * p + 2, 128:192])
                for h in range(8):
                    for n in range(NN):
                        ps = ps_proj.tile([128, CH], f32, tag="proj", name="proj")
                        for r in range(4):
                            nc.tensor.matmul(
                                ps, wqf[r][:, 192 * h:192 * h + 128],
                                xtq[r][:, CH * n:CH * (n + 1)],
                                start=(r == 0), stop=(r == 3))
                        nc.scalar.activation(
                            qz1[h][0:64, CH * n:CH * (n + 1)], ps[0:64, :],
                            AF.Identity, bias=bqp[0:64, h:h + 1])
                        nc.scalar.activation(
                            qz2[h][64:128, CH * n:CH * (n + 1)], ps[64:128, :],
                            AF.Identity, bias=bqp[64:128, h:h + 1])
                for p in range(4):
                    for n in range(NN):
                        ps = ps_proj.tile([128, CH], f32, tag="proj", name="proj")
                        for r in range(4):
                            nc.tensor.matmul(
                                ps, wgt[r][:, 128 * p:128 * (p + 1)],
                                xtq[r][:, CH * n:CH * (n + 1)],
                                start=(r == 0), stop=(r == 3))
                        nc.scalar.activation(
                            gt[p][:, CH * n:CH * (n + 1)], ps, AF.Identity,
                            bias=bg[:, p:p + 1])

                # --- deferred scalar/stat constants (off the critical path) ---
                lam128 = consts.tile([128, 1], f32, tag="lam128", name="lam128")
                nc.gpsimd.dma_start(out=lam128, in_=lam_d[:].to_broadcast([128, 1]))
                li128 = consts.tile([128, 1], f32, tag="li128", name="li128")
                nc.gpsimd.dma_start(out=li128, in_=li_d[:].to_broadcast([128, 1]))
                neglam = consts.tile([128, 1], f32, tag="neglam", name="neglam")
                ts_(neglam, lam128, -1.0, None, OP.mult)
                onelam = consts.tile([128, 1], f32, tag="onelam", name="onelam")
                ts_(onelam, lam128, -1.0, 1.0, OP.mult, OP.add)   # 1 - lam
                halfli = consts.tile([128, 1], f32, tag="halfli", name="halfli")
                ts_(halfli, li128, -0.5, 0.5, OP.mult, OP.add)    # 0.5*(1-li)

                bkp = consts.tile([128, 8], f32, tag="bkp", name="bkp")
                nc.sync.dma_start(
                    out=bkp,
                    in_=bk_d[:].rearrange("(h blk) -> blk h", blk=128))

                gamma128 = consts.tile([128, 1], f32, tag="gamma128", name="gamma128")
                nc.sync.dma_start(out=gamma128[0:64, :], in_=gamma_d[:])
                nc.sync.dma_start(out=gamma128[64:128, :], in_=gamma_d[:])
                beta128 = consts.tile([128, 1], f32, tag="beta128", name="beta128")
                nc.sync.dma_start(out=beta128[0:64, :], in_=beta_d[:])
                nc.sync.dma_start(out=beta128[64:128, :], in_=beta_d[:])
                bb128 = consts.tile([128, 1], f32, tag="bb128", name="bb128")
                ts_(bb128, beta128, halfli, None, OP.mult)        # beta*0.5*(1-li)

                # v-bias columns: head-major [64,8] for the stats corrections,
                # pair-stacked [128,4] for the final affine
                bvc = consts.tile([64, 8], f32, tag="bvc", name="bvc")
                nc.sync.dma_start(
                    out=bvc, in_=bv_d[:].rearrange("(h d) -> d h", d=64))
                cc64 = consts.tile([64, 8], f32, tag="cc64", name="cc64")
                ts_(cc64, bvc, onelam[0:64, :], None, OP.mult)
                bvc128 = consts.tile([128, 4], f32, tag="bvc128", name="bvc128")
                nc.sync.dma_start(
                    out=bvc128, in_=bv_d[:].rearrange("(p k d) -> (k d) p", k=2, d=64))
                cc128 = consts.tile([128, 4], f32, tag="cc128", name="cc128")
                ts_(cc128, bvc128, onelam, None, OP.mult)
                # cc-only GroupNorm stat corrections (ready before the tail)
                csq64 = consts.tile([64, 1], f32, tag="csq64", name="csq64")
                csum64 = consts.tile([64, 1], f32, tag="csum64", name="csum64")
                ccsq = consts.tile([64, 8], f32, tag="ccsq", name="ccsq")
                nc.vector.tensor_mul(ccsq, cc64, cc64)
                nc.vector.tensor_reduce(csq64, ccsq, axis=AX.X, op=OP.add)
                nc.vector.tensor_reduce(csum64, cc64, axis=AX.X, op=OP.add)

                # group matrix for the stats matmul, duplicated across both
                # 64-row halves: ind2b[d, d'] = 1 iff d//8 == (d' mod 64)//8
                ind2b = consts.tile([64, 128], f32, tag="ind2b", name="ind2b")
                nc.gpsimd.memset(ind2b, 1.0)
                nc.gpsimd.affine_select(
                    out=ind2b, in_=ind2b, compare_op=OP.is_ge, fill=0.0,
                    base=0, pattern=[[0, 2], [-8, 8], [0, 8]], channel_multiplier=1)
                nc.gpsimd.affine_select(
                    out=ind2b, in_=ind2b, compare_op=OP.is_ge, fill=0.0,
                    base=7, pattern=[[0, 2], [8, 8], [0, 8]], channel_multiplier=-1)

                # selector for the last half's PE-broadcast of the softmax
                # normalizers: sel2[r, x] = 1 iff x//64 == r   (r in 0..1)
                # bf16 so the broadcast matmul runs at full (non-fp32) rate
                sel2 = consts.tile([2, 128], bf16, tag="sel2", name="sel2")
                nc.gpsimd.memset(sel2, 1.0)
                nc.gpsimd.affine_select(
                    out=sel2, in_=sel2, compare_op=OP.is_ge, fill=0.0,
                    base=0, pattern=[[1, 128]], channel_multiplier=-64)
                nc.gpsimd.affine_select(
                    out=sel2, in_=sel2, compare_op=OP.is_ge, fill=0.0,
                    base=63, pattern=[[-1, 128]], channel_multiplier=64)

                # --- key path ---
                xtk = transpose_input(k_d)
                for r in range(4):
                    nc.sync.dma_start(out=wst[r][:, 0:2 * H * D],
                                      in_=wk_d[128 * r:128 * (r + 1), :])
                    nc.vector.tensor_copy(wkf[r], wst[r][:, 0:2 * H * D])
                for h in range(8):
                    for n in range(NN):
                        ps = ps_proj.tile([128, CH], f32, tag="proj", name="proj")
                        for r in range(4):
                            nc.tensor.matmul(
                                ps, wkf[r][:, 128 * h:128 * (h + 1)],
                                xtk[r][:, CH * n:CH * (n + 1)],
                                start=(r == 0), stop=(r == 3))
                        nc.scalar.activation(
                            kk[h][:, CH * n:CH * (n + 1)], ps,
                            AF.Identity, bias=bkp[:, h:h + 1])

                # --- values path (q-major, interleaved into v_aug + ones) ---
                xtv = transpose_input(v_d)
                for r in range(4):
                    nc.sync.dma_start(out=wst[r][:, 0:H * D],
                                      in_=wv_d[128 * r:128 * (r + 1), :])
                    nc.vector.tensor_copy(wvf[r], wst[r][:, 0:H * D])
                for i in range(NJ):
                    ps = ps_proj.tile([128, 512], f32, tag="proj", name="proj")
                    for r in range(4):
                        nc.tensor.matmul(
                            ps, xtv[r][:, 128 * i:128 * (i + 1)], wvf[r],
                            start=(r == 0), stop=(r == 3))
                    nc.vector.tensor_copy(
                        va[i][:, :, 0:64],
                        ps.rearrange("p (h d) -> p h d", d=64))
                    nc.gpsimd.memset(va[i][:, :, 64:65], 1.0)

                # gate tanh now (ACT is free here); bgt = tanh(g/2) + 1  (bf16)
                for p in range(4):
                    th = thp.tile([128, S], f32, tag="th", name="th")
                    nc.scalar.activation(th, gt[p], AF.Tanh, scale=0.5)
                    ts_(bgt[p], th, 1.0, None, OP.add)



            # ---------- phase 2: attention per head (pairs for epilogue) ----
            with tc.tile_pool(name="ps_att", bufs=2, space="PSUM") as ps_att, \
                 tc.tile_pool(name="ps_o", bufs=2, space="PSUM") as ps_o, \
                 tc.tile_pool(name="expp", bufs=3) as expp, \
                 tc.tile_pool(name="osp", bufs=2) as osp, \
                 tc.tile_pool(name="typ", bufs=2) as typ, \
                 tc.tile_pool(name="tailp", bufs=1) as tailp, \
                 tc.tile_pool(name="oq", bufs=3) as oqp, \
                 tc.tile_pool(name="spp", bufs=2) as spp:

                ty3 = None
                for p in range(4):
                    ty = typ.tile([128, S], f32, tag="ty", name="ty")
                    for half in range(2):
                        h = 2 * p + half
                        last = (p == 3 and half == 1)
                        prow = 64 * half   # q1/k1 in rows 0-63, q2/k2 in 64-127
                        sp2 = spp.tile([2, S], f32, tag="sp2", name="sp2")
                        os_c = {}
                        # term-sequential: only one o accumulator lives at a
                        # time, so both the score tiles and the o tiles can
                        # double-buffer inside the 8-bank PSUM budget.
                        for t, qz_ in ((1, qz1), (2, qz2)):
                            o_ps = ps_o.tile([65, S], f32, tag="o", name="o")
                            for j in range(NJ):
                                s_ps = ps_att.tile([128, S], f32, tag="s", name="s")
                                for n in range(NN):
                                    nc.tensor.matmul(
                                        s_ps[:, CH * n:CH * (n + 1)],
                                        kk[h][:, 128 * j:128 * (j + 1)],
                                        qz_[h][:, CH * n:CH * (n + 1)],
                                        start=True, stop=True)
                                ex = expp.tile([128, S], bf16, tag="exp", name="exp")
                                nc.scalar.activation(ex, s_ps, AF.Exp, scale=INV)
                                for n in range(NN):
                                    nc.tensor.matmul(
                                        o_ps[:, CH * n:CH * (n + 1)],
                                        va[j][:, h, :],
                                        ex[:, CH * n:CH * (n + 1)],
                                        start=(j == 0), stop=(j == NJ - 1))
                            os_ = osp.tile([65, S], f32, tag=f"os{t}", name=f"os{t}")
                            if last:
                                # ACT is idle once the exps are done; freeing
                                # the DVE queue for the combine chain
                                nc.scalar.copy(os_, o_ps)
                            else:
                                nc.vector.tensor_copy(os_, o_ps)
                            nc.sync.dma_start(
                                out=sp2[t - 1:t, :], in_=os_[64:65, :])
                            os_c[t] = os_

                        # per-half combine: softmax-normalize, subtract the
                        # lam-weighted term, accumulate GN stats
                        rp2 = spp.tile([2, S], f32, tag="rp2", name="rp2")
                        if last:
                            # tail: fast reciprocal (~1e-3 rel err on two
                            # heads' normalizers, well inside tolerance) and
                            # a selector-matmul broadcast on the idle PE
                            nc.vector.reciprocal_approx_fast(out=rp2, in_=sp2)
                            rpb = spp.tile([2, S], bf16, tag="rpb", name="rpb")
                            nc.vector.tensor_copy(rpb, rp2)
                            bc = ps_att.tile([128, S], f32, tag="s", name="bcpe")
                            for n in range(NN):
                                nc.tensor.matmul(
                                    bc[:, CH * n:CH * (n + 1)], sel2,
                                    rpb[:, CH * n:CH * (n + 1)],
                                    start=True, stop=True)
                            bcs1, bcs2 = bc[0:64, :], bc[64:128, :]
                        else:
                            nc.vector.reciprocal_approx_fast(out=rp2, in_=sp2)
                            bcs1 = spp.tile([64, S], f32, tag="bcs1", name="bcs1")
                            bcs2 = spp.tile([64, S], f32, tag="bcs2", name="bcs2")
                            r1 = spp.tile([1, S], f32, tag="rst1", name="rst1")
                            nc.sync.dma_start(out=r1, in_=rp2[0:1, :])
                            r2 = spp.tile([1, S], f32, tag="rst2", name="rst2")
                            nc.sync.dma_start(out=r2, in_=rp2[1:2, :])
                            nc.gpsimd.partition_broadcast(bcs1, r1[0:1, :], channels=64)
                            nc.gpsimd.partition_broadcast(bcs2, r2[0:1, :], channels=64)
                        os1, os2 = os_c[1], os_c[2]
                        nc.vector.tensor_mul(os1[0:64, :], os1[0:64, :], bcs1)
                        stt(os2[0:64, :], os2[0:64, :], neglam[0:64, :], bcs2,
                            OP.mult, OP.mult)
                        tyh = ty[prow:prow + 64, :]
                        stt(tyh, os1[0:64, :], 1.0, os2[0:64, :],
                            OP.bypass, OP.add, accum_out=sumcol[:, h:h + 1])
                        stt(os1[0:64, :], tyh, 1.0, tyh, OP.mult, OP.mult,
                            accum_out=sumcol[:, 8 + h:9 + h])
                    # yg = y * (tanh+1)  (bf16, consumed by the fused
                    # scale+transpose in phase 3).  The last pair's is
                    # deferred into the stats chain so the reductions start
                    # immediately after its ysq.
                    if p < 3:
                        nc.vector.tensor_mul(ypair[p], ty, bgt[p])
                    else:
                        ty3 = ty

                # ------- tail: stats + fused scale/transpose output -------
                # (same pool scope: ty3 must stay alive, and the stats/output
                # PSUM reuses the attention pools' banks)
                tot = tailp.tile([64, 2], f32, tag="tot", name="tot")
                nc.vector.tensor_reduce(
                    tot, sumcol.rearrange("p (t h) -> p t h", h=8),
                    axis=AX.X, op=OP.add)
                # bias-C (bv) corrections to the raw-Y stats
                csc = tailp.tile([64, 8], f32, tag="csc", name="csc")
                nc.vector.tensor_mul(csc, cc64, sumcol[:, 0:8])
                cy64 = tailp.tile([64, 1], f32, tag="cy64", name="cy64")
                nc.vector.tensor_reduce(cy64, csc, axis=AX.X, op=OP.add)
                tot2 = tailp.tile([64, 2], f32, tag="tot2", name="tot2")
                stt(tot2[:, 0:1], csum64, float(S), tot[:, 0:1], OP.mult, OP.add)
                stt(tot2[:, 1:2], cy64, 2.0, tot[:, 1:2], OP.mult, OP.add)
                stt(tot2[:, 1:2], csq64, float(S), tot2[:, 1:2], OP.mult, OP.add)

                ms_ps = ps_o.tile([128, 2], f32, tag="o", name="ms")
                nc.tensor.matmul(ms_ps, ind2b, tot2, start=True, stop=True)
                mean = tailp.tile([128, 1], f32, tag="mean", name="mean")
                ts_(mean, ms_ps[:, 0:1], 1.0 / CNT, None, OP.mult)
                e2 = tailp.tile([128, 1], f32, tag="e2", name="e2")
                ts_(e2, ms_ps[:, 1:2], 1.0 / CNT, None, OP.mult)
                nm2 = tailp.tile([128, 1], f32, tag="nm2", name="nm2")
                ts_(nm2, mean, mean, -1.0, OP.mult, OP.mult)
                veps = tailp.tile([128, 1], f32, tag="veps", name="veps")
                stt(veps, nm2, EPS, e2, OP.add, OP.add)
                # deferred last-pair gate fold: slots into the DVE queue while
                # the scalar engine loads the sqrt table set
                nc.vector.tensor_mul(ypair[3], ty3, bgt[3])
                sd = tailp.tile([128, 1], f32, tag="sd", name="sd")
                nc.scalar.activation(sd, veps, AF.Sqrt)
                rsd = tailp.tile([128, 1], f32, tag="rsd", name="rsd")
                nc.vector.reciprocal(rsd, sd)
                # one Newton step for rsqrt accuracy (ACT sqrt is loose)
                rr = tailp.tile([128, 1], f32, tag="rr", name="rr")
                nc.vector.tensor_mul(rr, rsd, rsd)
                nc.vector.tensor_mul(rr, rr, veps)
                ts_(rr, rr, -0.5, 1.5, OP.mult, OP.add)
                rstd = tailp.tile([128, 1], f32, tag="rstd", name="rstd")
                nc.vector.tensor_mul(rstd, rsd, rr)

                a128 = tailp.tile([128, 1], f32, tag="a128", name="a128")
                ts_(a128, rstd, gamma128, halfli, OP.mult, OP.mult)
                cm128 = tailp.tile([128, 4], f32, tag="cm128", name="cm128")
                ts_(cm128, cc128, mean, None, OP.subtract)
                ball = tailp.tile([128, 4], f32, tag="ball", name="ball")
                ts_(ball, cm128, a128, bb128, OP.mult, OP.add)

                # scaled-identity matrices: out = yg^T diag(a) + bgt^T diag(ball)
                da = tailp.tile([128, 128], bf16, tag="da", name="da")
                ts_(da, ident_b, a128, None, OP.mult)
                db = []
                for p in range(4):
                    d_t = tailp.tile([128, 128], bf16, tag=f"db{p}", name=f"db{p}")
                    ts_(d_t, ident_b, ball[:, p:p + 1], None, OP.mult)
                    db.append(d_t)

                for c in range(NJ):
                    tp_o = ps_att.tile([128, 512], f32, tag="s", name="tp_out")
                    for p in range(4):
                        nc.tensor.matmul(
                            tp_o[:, 128 * p:128 * (p + 1)],
                            ypair[p][:, 128 * c:128 * (c + 1)], da,
                            start=True, stop=False)
                        nc.tensor.matmul(
                            tp_o[:, 128 * p:128 * (p + 1)],
                            bgt[p][:, 128 * c:128 * (c + 1)], db[p],
                            start=False, stop=True)
                    oq = oqp.tile([128, 512], f32, tag="oq", name="oq")
                    nc.scalar.copy(oq, tp_o)
                    nc.sync.dma_start(out=out_d[128 * c:128 * (c + 1), :], in_=oq)

    nc.finalize()
    return nc


_CACHE = {}


def _get_nc():
    if "nc" not in _CACHE:
        _CACHE["nc"] = build_nc(S_FULL)
    return _CACHE["nc"]


def run(inputs, trace=False, tmpdir=None):
    from concourse.bass_utils import run_bass_kernel_spmd
    nc = _get_nc()
    arrs = {k: np.asarray(v, dtype=np.float32) for k, v in inputs.items()}
    shared = {k: np.ascontiguousarray(arrs[k]) for k in
              ("Wq", "bq", "Wk", "bk", "Wv", "bv", "gamma", "beta",
               "lam", "lambda_init")}
    in_maps = []
    for i in range(B):
        m = dict(shared)
        m["query"] = np.ascontiguousarray(arrs["query"][i])
        m["key"] = np.ascontiguousarray(arrs["key"][i])
        m["values"] = np.ascontiguousarray(arrs["values"][i])
        in_maps.append(m)
    res = run_bass_kernel_spmd(nc, in_maps, core_ids=list(range(B)),
                               trace=trace, tmpdir=tmpdir)
    out = np.stack([res.results[i]["out"] for i in range(B)], axis=0)
    return out.astype(np.float32), res


def kernel(**inputs):
    out, _ = run(inputs)
    return out
